# revision 3
# baseline (speedup 1.0000x reference)
"""ContextConditionedAttention Trainium2 kernel.

Full-input contract: kernel(**inputs) takes the unsharded numpy inputs and
returns the full (B, N, HIDDEN) float32 output. Internally the work is
sharded over 8 NeuronCores as (batch b in 0..3) x (head-group g in 0..1),
4 heads per core. Each core computes its head-group's partial out-projection
(2048, 512); the host sums the two head-group partials per batch and adds
the bias epilogue.

Math notes (exact simplifications vs the reference):
  - per-(batch,head) softmax bias bias_emb[ct] is constant along the softmax
    axis -> cancels in softmax -> dropped.
  - keymod_emb[ct] adds to K -> folded into the K projection bias.
  - attn_mask folds into the exp() activation as a per-key additive bias
    (0 or -1e30).
  - V bias + out bias: softmax rows sum to 1 -> P@(V + 1 bv^T) = P@V + 1 bv^T,
    so host epilogue adds (bv @ wo.T + bo).

On-chip layout (per core): everything is computed transposed so no on-chip
transposes are needed:
  Q^T/K^T (d on partitions, tokens free) from lhsT=wq^T chunks, rhs=x^T;
  S^T = K @ Q^T (keys on partitions); P^T = exp(S^T/8 + mask);
  O^T = [V | 1]^T-style matmul: lhsT=(keys,65) with a ones column giving the
  softmax denominator in row 64; out-proj from lhsT=O^T blocks.
"""

import os
import numpy as np
import ml_dtypes

B, N, HIDDEN = 4, 2048, 512
N_HEADS, HEAD_DIM = 8, 64
G_HEADS = 4          # heads per core (head-group)
G_DIM = 256          # dims per head-group
N_CORES = 8
NKB = N // 128       # key blocks of 128
NQB = N // 128       # query blocks of 128
QCH = 1024           # query chunk for the attention inner loop
SCALE = 1.0 / float(np.sqrt(HEAD_DIM))
MASK_NEG = -1.0e30

BF16 = ml_dtypes.bfloat16

_CACHE = {}


def _build_program():
    import concourse.bacc as bacc
    import concourse.mybir as mybir
    import concourse.tile as tile

    nc = bacc.Bacc("TRN2", target_bir_lowering=False, debug=False,
                   num_devices=N_CORES)
    f32 = mybir.dt.float32
    bf16 = mybir.dt.bfloat16

    # DRAM I/O (per-core shards; same program on all 8 cores)
    xt_d = nc.dram_tensor("xt", (HIDDEN, N), bf16, kind="ExternalInput").ap()
    wq_d = nc.dram_tensor("wq", (4, 128, G_DIM), bf16, kind="ExternalInput").ap()
    wk_d = nc.dram_tensor("wk", (4, 128, G_DIM), bf16, kind="ExternalInput").ap()
    wv_d = nc.dram_tensor("wv", (4, 128, G_DIM), bf16, kind="ExternalInput").ap()
    wo_d = nc.dram_tensor("wo", (2, 128, HIDDEN), bf16, kind="ExternalInput").ap()
    qb_d = nc.dram_tensor("qb", (2, 128, 1), f32, kind="ExternalInput").ap()
    kb_d = nc.dram_tensor("kb", (2, 128, 1), f32, kind="ExternalInput").ap()
    mk_d = nc.dram_tensor("mk", (NKB, 128, 1), f32, kind="ExternalInput").ap()
    y_d = nc.dram_tensor("y", (N, HIDDEN), f32, kind="ExternalOutput").ap()

    with tile.TileContext(nc) as tc:
        with tc.tile_pool(name="sb", bufs=1) as sb, \
             tc.tile_pool(name="pp", bufs=2) as pp:
            # ---- persistent SBUF tiles ----
            xt_sb = [sb.tile([128, N], bf16, tag=f"xt{c}", name=f"xt{c}") for c in range(4)]
            wq_sb = sb.tile([128, 4, G_DIM], bf16, tag="wq", name="wq_sb")
            wk_sb = sb.tile([128, 4, G_DIM], bf16, tag="wk", name="wk_sb")
            wv_sb = sb.tile([128, 4, G_DIM], bf16, tag="wv", name="wv_sb")
            wo_sb = sb.tile([128, 2, HIDDEN], bf16, tag="wo", name="wo_sb")
            qb_sb = sb.tile([128, 2], f32, tag="qb", name="qb_sb")
            kb_sb = sb.tile([128, 2], f32, tag="kb", name="kb_sb")
            mk_sb = sb.tile([128, NKB], f32, tag="mk", name="mk_sb")
            qt_sb = [sb.tile([128, N], bf16, tag=f"qt{hp}", name=f"qt{hp}") for hp in range(2)]
            kt_sb = [sb.tile([128, N], bf16, tag=f"kt{hp}", name=f"kt{hp}") for hp in range(2)]
            # V with a ones column per (key-block, head): (128, kb, h, 65)
            v_sb = sb.tile([128, NKB, G_HEADS, HEAD_DIM + 1], bf16, tag="v", name="v_sb")
            ot_sb = [sb.tile([128, N], bf16, tag=f"ot{hp}", name=f"ot{hp}") for hp in range(2)]
            ones_sb = sb.tile([1, HEAD_DIM], f32, tag="ones", name="ones_sb")
            r_sb = [sb.tile([1, QCH], f32, tag=f"r{i}", name=f"r{i}") for i in range(2)]

            # ---- input DMAs ----
            for c in range(4):
                nc.sync.dma_start(xt_sb[c][:], xt_d[c * 128:(c + 1) * 128, :])
            for c in range(4):
                nc.sync.dma_start(wq_sb[:, c, :], wq_d[c])
                nc.sync.dma_start(wk_sb[:, c, :], wk_d[c])
                nc.sync.dma_start(wv_sb[:, c, :], wv_d[c])
            for hp in range(2):
                nc.sync.dma_start(wo_sb[:, hp, :], wo_d[hp])
                nc.sync.dma_start(qb_sb[:, hp:hp + 1], qb_d[hp])
                nc.sync.dma_start(kb_sb[:, hp:hp + 1], kb_d[hp])
            for k in range(NKB):
                nc.sync.dma_start(mk_sb[:, k:k + 1], mk_d[k])
            nc.vector.memset(v_sb[:], 1.0)
            nc.vector.memset(ones_sb[:], 1.0)

            # ---- phase 1: projections ----
            with tc.tile_pool(name="pj", bufs=3, space="PSUM") as pj, \
                 tc.tile_pool(name="pv", bufs=2, space="PSUM") as pv:
                for hp in range(2):
                    for (w_sb, bias_sb, dst) in ((wq_sb, qb_sb, qt_sb),
                                                 (wk_sb, kb_sb, kt_sb)):
                        for half in range(2):
                            ps = pj.tile([128, QCH], f32, tag="qk", name="ps_qk")
                            for c in range(4):
                                for s in range(2):
                                    fr = half * QCH + s * 512
                                    nc.tensor.matmul(
                                        ps[:, s * 512:(s + 1) * 512],
                                        lhsT=w_sb[:, c, hp * 128:(hp + 1) * 128],
                                        rhs=xt_sb[c][:, fr:fr + 512],
                                        start=(c == 0), stop=(c == 3))
                            nc.vector.tensor_scalar_add(
                                dst[hp][:, half * QCH:(half + 1) * QCH],
                                ps[:], bias_sb[:, hp:hp + 1])
                    # V for this hp is shared; compute V once (hp==0)
                    if hp == 0:
                        for k in range(NKB):
                            psv = pv.tile([128, G_DIM], f32, tag="v", name="ps_v")
                            for c in range(4):
                                nc.tensor.matmul(
                                    psv[:],
                                    lhsT=xt_sb[c][:, k * 128:(k + 1) * 128],
                                    rhs=wv_sb[:, c, :],
                                    start=(c == 0), stop=(c == 3))
                            nc.vector.tensor_copy(
                                v_sb[:, k, :, 0:HEAD_DIM],
                                psv.rearrange("p (h d) -> p h d", h=G_HEADS))

            # ---- phase 2: attention ----
            Exp = mybir.ActivationFunctionType.Exp
            with tc.tile_pool(name="at", bufs=1, space="PSUM") as at:
                for hp in range(2):
                    for qc in range(2):
                        q0 = qc * QCH
                        a_ps = [at.tile([128, QCH], f32, tag=f"a{i}", name=f"ps_a{i}")
                                for i in range(2)]
                        b_ps = [at.tile([HEAD_DIM + 1, QCH], f32, tag=f"b{i}", name=f"ps_b{i}")
                                for i in range(2)]
                        for k in range(NKB):
                            p_t = [pp.tile([128, QCH], bf16, tag=f"p{i}", name=f"p{i}")
                                   for i in range(2)]
                            for i in range(2):   # i = head within pair
                                r0 = i * 64
                                for s in range(2):
                                    nc.tensor.matmul(
                                        a_ps[i][:, s * 512:(s + 1) * 512],
                                        lhsT=kt_sb[hp][r0:r0 + 64,
                                                       k * 128:(k + 1) * 128],
                                        rhs=qt_sb[hp][r0:r0 + 64,
                                                      q0 + s * 512:q0 + (s + 1) * 512],
                                        start=True, stop=True)
                            for i in range(2):
                                nc.scalar.activation(
                                    p_t[i][:], a_ps[i][:], Exp,
                                    bias=mk_sb[:, k:k + 1], scale=SCALE)
                            for i in range(2):
                                h = 2 * hp + i
                                for s in range(2):
                                    nc.tensor.matmul(
                                        b_ps[i][:, s * 512:(s + 1) * 512],
                                        lhsT=v_sb[:, k, h, :],
                                        rhs=p_t[i][:, s * 512:(s + 1) * 512],
                                        start=(k == 0), stop=(k == NKB - 1))
                        # normalize: recip of denominators, broadcast, multiply
                        for i in range(2):
                            nc.vector.reciprocal(
                                r_sb[i][:], b_ps[i][HEAD_DIM:HEAD_DIM + 1, :])
                        for i in range(2):
                            rb = at.tile([HEAD_DIM, QCH], f32, tag=f"a{i}", name=f"ps_rb{i}")
                            for s in range(2):
                                nc.tensor.matmul(
                                    rb[:, s * 512:(s + 1) * 512],
                                    lhsT=ones_sb[:],
                                    rhs=r_sb[i][:, s * 512:(s + 1) * 512],
                                    start=True, stop=True)
                            # DVE has one PSUM read port: stage rb in SBUF
                            rbs = pp.tile([HEAD_DIM, QCH], f32, tag=f"rbs{i}",
                                          name=f"rbs{i}")
                            nc.vector.tensor_copy(rbs[:], rb[:])
                            nc.vector.tensor_mul(
                                ot_sb[hp][i * 64:(i + 1) * 64, q0:q0 + QCH],
                                b_ps[i][0:HEAD_DIM, :], rbs[:])

            # ---- phase 3: out-projection (partial over this head-group) ----
            with tc.tile_pool(name="yo", bufs=2, space="PSUM") as yo, \
                 tc.tile_pool(name="ys", bufs=2) as ys:
                for qb in range(NQB):
                    yp = yo.tile([128, HIDDEN], f32, tag="y", name="ps_y")
                    for hp in range(2):
                        nc.tensor.matmul(
                            yp[:],
                            lhsT=ot_sb[hp][:, qb * 128:(qb + 1) * 128],
                            rhs=wo_sb[:, hp, :],
                            start=(hp == 0), stop=(hp == 1))
                    yt = ys.tile([128, HIDDEN], f32, tag="yt", name="yt")
                    nc.vector.tensor_copy(yt[:], yp[:])
                    nc.sync.dma_start(y_d[qb * 128:(qb + 1) * 128, :], yt[:])

    nc.compile()
    return nc


def _get_program():
    if "nc" not in _CACHE:
        _CACHE["nc"] = _build_program()
    return _CACHE["nc"]


def _prep_inputs(x, cancer_type, attn_mask, wq, bq, wk, bk, wv, bv, wo, bo,
                 bias_emb, keymod_emb):
    """Host-side shard prep: returns (in_maps list of 8, epilogue (B,512))."""
    x = np.asarray(x, dtype=np.float32)
    ct = np.asarray(cancer_type).astype(np.int64)
    mask = np.asarray(attn_mask)
    wq = np.asarray(wq, dtype=np.float32)
    wk = np.asarray(wk, dtype=np.float32)
    wv = np.asarray(wv, dtype=np.float32)
    wo = np.asarray(wo, dtype=np.float32)
    bq = np.asarray(bq, dtype=np.float32)
    bk = np.asarray(bk, dtype=np.float32)
    bv = np.asarray(bv, dtype=np.float32)
    bo = np.asarray(bo, dtype=np.float32)
    keymod = np.asarray(keymod_emb, dtype=np.float32)

    wqt = np.ascontiguousarray(wq.T).astype(BF16)     # (in 512, out 512)
    wkt = np.ascontiguousarray(wk.T).astype(BF16)
    wvt = np.ascontiguousarray(wv.T).astype(BF16)
    wot = np.ascontiguousarray(wo.T).astype(BF16)

    xt_all = [np.ascontiguousarray(x[b].T).astype(BF16) for b in range(B)]
    mka = np.where(mask, np.float32(MASK_NEG), np.float32(0.0)).astype(np.float32)

    in_maps = []
    for core in range(N_CORES):
        b, g = core // 2, core % 2
        gs = slice(g * G_DIM, (g + 1) * G_DIM)
        kbias = (bk + keymod[ct[b]])[gs].reshape(2, 128, 1).astype(np.float32)
        qbias = bq[gs].reshape(2, 128, 1).astype(np.float32)
        in_maps.append({
            "xt": xt_all[b],
            "wq": np.ascontiguousarray(wqt[:, gs].reshape(4, 128, G_DIM)),
            "wk": np.ascontiguousarray(wkt[:, gs].reshape(4, 128, G_DIM)),
            "wv": np.ascontiguousarray(wvt[:, gs].reshape(4, 128, G_DIM)),
            "wo": np.ascontiguousarray(wot[gs, :].reshape(2, 128, HIDDEN)),
            "qb": qbias,
            "kb": kbias,
            "mk": np.ascontiguousarray(mka[b].reshape(NKB, 128, 1)),
        })
    epilogue = (bv @ wo.T + bo).astype(np.float32)    # (512,)
    return in_maps, epilogue


def kernel(**inputs):
    from concourse import bass_utils

    nc = _get_program()
    in_maps, epilogue = _prep_inputs(**inputs)
    res = bass_utils.run_bass_kernel_spmd(nc, in_maps,
                                          core_ids=list(range(N_CORES)))
    out = np.empty((B, N, HIDDEN), dtype=np.float32)
    for b in range(B):
        out[b] = res.results[2 * b]["y"] + res.results[2 * b + 1]["y"] + epilogue
    return out


# revision 4
# speedup vs baseline: 1.0972x; 1.0972x over previous
"""ContextConditionedAttention Trainium2 kernel.

Full-input contract: kernel(**inputs) takes the unsharded numpy inputs and
returns the full (B, N, HIDDEN) float32 output. Internally the work is
sharded over 8 NeuronCores as (batch b in 0..3) x (head-group g in 0..1),
4 heads per core. Each core computes its head-group's partial out-projection
(2048, 512); the host sums the two head-group partials per batch and adds
the bias epilogue.

Math notes (exact simplifications vs the reference):
  - per-(batch,head) softmax bias bias_emb[ct] is constant along the softmax
    axis -> cancels in softmax -> dropped.
  - keymod_emb[ct] adds to K -> folded into the K projection bias.
  - attn_mask folds into the exp() activation as a per-key additive bias
    (0 or -1e30).
  - V bias + out bias: softmax rows sum to 1 -> P@(V + 1 bv^T) = P@V + 1 bv^T,
    so host epilogue adds (bv @ wo.T + bo).

On-chip layout (per core): everything is computed transposed so no on-chip
transposes are needed:
  Q^T/K^T (d on partitions, tokens free) from lhsT=wq^T chunks, rhs=x^T;
  S^T = K @ Q^T (keys on partitions, row-tiled head pairs); P^T = exp(S^T/8
  + mask) on the scalar engine; O^T accumulated from lhsT=[V | 1] (the ones
  column yields the softmax denominator in row 64); normalization via DVE
  reciprocal + GPSIMD partition_broadcast; out-proj from lhsT=O^T blocks,
  emitted per query-chunk so the store overlaps the next chunk's attention.

PSUM (8 banks total) is a single pool: tags a0/a1 (128,1024 f32, 2 banks
each) shared by Q/K projections, score tiles, and the normalize broadcast;
tags b0/b1 (65,1024 f32, 2 banks each) shared by V projection, O^T
accumulators, and the out-projection tiles.
"""

import numpy as np
import ml_dtypes

B, N, HIDDEN = 4, 2048, 512
N_HEADS, HEAD_DIM = 8, 64
G_HEADS = 4          # heads per core (head-group)
G_DIM = 256          # dims per head-group
N_CORES = 8
NKB = N // 128       # key blocks of 128
NQB = N // 128       # query blocks of 128
QCH = 1024           # query chunk for the attention inner loop
SCALE = 1.0 / float(np.sqrt(HEAD_DIM))
MASK_NEG = -1.0e30

BF16 = ml_dtypes.bfloat16

_CACHE = {}


def _build_program():
    import concourse.bacc as bacc
    import concourse.mybir as mybir
    import concourse.tile as tile
    from concourse import bass_isa

    nc = bacc.Bacc("TRN2", target_bir_lowering=False, debug=False,
                   num_devices=N_CORES)
    f32 = mybir.dt.float32
    bf16 = mybir.dt.bfloat16
    Exp = mybir.ActivationFunctionType.Exp

    # DRAM I/O (per-core shards; same program on all 8 cores)
    xt_d = nc.dram_tensor("xt", (HIDDEN, N), bf16, kind="ExternalInput").ap()
    wq_d = nc.dram_tensor("wq", (4, 128, G_DIM), bf16, kind="ExternalInput").ap()
    wk_d = nc.dram_tensor("wk", (4, 128, G_DIM), bf16, kind="ExternalInput").ap()
    wv_d = nc.dram_tensor("wv", (4, 128, G_DIM), bf16, kind="ExternalInput").ap()
    wo_d = nc.dram_tensor("wo", (2, 128, HIDDEN), bf16, kind="ExternalInput").ap()
    qb_d = nc.dram_tensor("qb", (2, 128, 1), f32, kind="ExternalInput").ap()
    kb_d = nc.dram_tensor("kb", (2, 128, 1), f32, kind="ExternalInput").ap()
    mk_d = nc.dram_tensor("mk", (NKB, 128, 1), f32, kind="ExternalInput").ap()
    y_d = nc.dram_tensor("y", (N, HIDDEN), f32, kind="ExternalOutput").ap()

    with tile.TileContext(nc) as tc:
        with tc.tile_pool(name="sb", bufs=1) as sb, \
             tc.tile_pool(name="pp", bufs=2) as pp, \
             tc.tile_pool(name="ps", bufs=1, space="PSUM") as ps:
            # ---- persistent SBUF tiles ----
            xt_sb = [sb.tile([128, N], bf16, tag=f"xt{c}", name=f"xt{c}")
                     for c in range(4)]
            wq_sb = sb.tile([128, 4, G_DIM], bf16, tag="wq", name="wq_sb")
            wk_sb = sb.tile([128, 4, G_DIM], bf16, tag="wk", name="wk_sb")
            wv_sb = sb.tile([128, 4, G_DIM], bf16, tag="wv", name="wv_sb")
            wo_sb = sb.tile([128, 2, HIDDEN], bf16, tag="wo", name="wo_sb")
            qb_sb = sb.tile([128, 2], f32, tag="qb", name="qb_sb")
            kb_sb = sb.tile([128, 2], f32, tag="kb", name="kb_sb")
            mk_sb = sb.tile([128, NKB], f32, tag="mk", name="mk_sb")
            qt_sb = [sb.tile([128, N], bf16, tag=f"qt{hp}", name=f"qt{hp}")
                     for hp in range(2)]
            kt_sb = [sb.tile([128, N], bf16, tag=f"kt{hp}", name=f"kt{hp}")
                     for hp in range(2)]
            # V with a ones column per (key-block, head): (128, kb, h, 65)
            v_sb = sb.tile([128, NKB, G_HEADS, HEAD_DIM + 1], bf16, tag="v",
                           name="v_sb")
            ot_sb = [sb.tile([128, N], bf16, tag=f"ot{hp}", name=f"ot{hp}")
                     for hp in range(2)]

            # ---- input DMAs ----
            for c in range(4):
                nc.sync.dma_start(xt_sb[c][:], xt_d[c * 128:(c + 1) * 128, :])
            for c in range(4):
                nc.sync.dma_start(wq_sb[:, c, :], wq_d[c])
                nc.sync.dma_start(wk_sb[:, c, :], wk_d[c])
                nc.sync.dma_start(wv_sb[:, c, :], wv_d[c])
            for hp in range(2):
                nc.sync.dma_start(wo_sb[:, hp, :], wo_d[hp])
                nc.sync.dma_start(qb_sb[:, hp:hp + 1], qb_d[hp])
                nc.sync.dma_start(kb_sb[:, hp:hp + 1], kb_d[hp])
            for k in range(NKB):
                nc.sync.dma_start(mk_sb[:, k:k + 1], mk_d[k])
            nc.vector.memset(v_sb[:], 1.0)

            # ---- phase A: projections (psum tags shared with attention) ----
            vk = 0   # V key-block emission counter (interleaved with Q/K)

            def emit_v_block():
                nonlocal vk
                if vk >= NKB:
                    return
                k = vk
                psv = ps.tile([128, G_DIM], f32, tag=f"b{k % 2}",
                              name=f"ps_v{k}")
                for c in range(4):
                    nc.tensor.matmul(
                        psv[:],
                        lhsT=xt_sb[c][:, k * 128:(k + 1) * 128],
                        rhs=wv_sb[:, c, :],
                        start=(c == 0), stop=(c == 3))
                nc.vector.tensor_copy(
                    v_sb[:, k, :, 0:HEAD_DIM],
                    psv.rearrange("p (h d) -> p h d", h=G_HEADS))
                vk += 1

            for hp in range(2):
                for j, (w_sb, bias_sb, dst) in enumerate(
                        ((wq_sb, qb_sb, qt_sb), (wk_sb, kb_sb, kt_sb))):
                    for half in range(2):
                        pst = ps.tile([128, QCH], f32, tag=f"a{j}",
                                      name=f"ps_qk{hp}{j}{half}")
                        for c in range(4):
                            for s in range(2):
                                fr = half * QCH + s * 512
                                nc.tensor.matmul(
                                    pst[:, s * 512:(s + 1) * 512],
                                    lhsT=w_sb[:, c, hp * 128:(hp + 1) * 128],
                                    rhs=xt_sb[c][:, fr:fr + 512],
                                    start=(c == 0), stop=(c == 3))
                        nc.vector.tensor_scalar_add(
                            dst[hp][:, half * QCH:(half + 1) * QCH],
                            pst[:], bias_sb[:, hp:hp + 1])
                        emit_v_block()
                        emit_v_block()
            while vk < NKB:
                emit_v_block()

            # ---- phase B: attention + per-chunk out-projection ----
            for qc in range(2):
                q0 = qc * QCH
                for hp in range(2):
                    b_ps = [ps.tile([HEAD_DIM + 1, QCH], f32, tag=f"b{i}",
                                    name=f"ps_b{qc}{hp}{i}")
                            for i in range(2)]
                    for k in range(NKB):
                        a_ps = [ps.tile([128, QCH], f32, tag=f"a{i}",
                                        name=f"ps_a{qc}{hp}{k}{i}")
                                for i in range(2)]
                        p_t = [pp.tile([128, QCH], bf16, tag=f"p{i}",
                                       name=f"p{qc}{hp}{k}{i}")
                               for i in range(2)]
                        for i in range(2):   # i = head within pair
                            r0 = i * 64
                            for s in range(2):
                                nc.tensor.matmul(
                                    a_ps[i][:, s * 512:(s + 1) * 512],
                                    lhsT=kt_sb[hp][r0:r0 + 64,
                                                   k * 128:(k + 1) * 128],
                                    rhs=qt_sb[hp][r0:r0 + 64,
                                                  q0 + s * 512:q0 + (s + 1) * 512],
                                    start=True, stop=True)
                            nc.scalar.activation(
                                p_t[i][:], a_ps[i][:], Exp,
                                bias=mk_sb[:, k:k + 1], scale=SCALE)
                            h = 2 * hp + i
                            for s in range(2):
                                nc.tensor.matmul(
                                    b_ps[i][:, s * 512:(s + 1) * 512],
                                    lhsT=v_sb[:, k, h, :],
                                    rhs=p_t[i][:, s * 512:(s + 1) * 512],
                                    start=(k == 0), stop=(k == NKB - 1))
                    # normalize: reciprocal of denominators (row 64 of b),
                    # partition_broadcast on GPSIMD, multiply into O^T sbuf
                    for i in range(2):
                        r_t = pp.tile([1, QCH], f32, tag=f"r{i}",
                                      name=f"r{qc}{hp}{i}")
                        nc.vector.reciprocal(
                            r_t[:], b_ps[i][HEAD_DIM:HEAD_DIM + 1, :])
                        rb_t = pp.tile([HEAD_DIM, QCH], f32, tag=f"rb{i}",
                                       name=f"rb{qc}{hp}{i}")
                        nc.gpsimd.partition_broadcast(rb_t[:], r_t[:])
                        nc.vector.tensor_mul(
                            ot_sb[hp][i * 64:(i + 1) * 64, q0:q0 + QCH],
                            b_ps[i][0:HEAD_DIM, :], rb_t[:])
                # out-projection for this query chunk (b-slots are free now)
                with tc.tile_pool(name=f"ysb{qc}", bufs=2) as ys:
                    for j in range(NQB // 2):
                        qb = qc * (NQB // 2) + j
                        yp = ps.tile([128, HIDDEN], f32, tag=f"b{j % 2}",
                                     name=f"ps_y{qb}")
                        for hp in range(2):
                            nc.tensor.matmul(
                                yp[:],
                                lhsT=ot_sb[hp][:, qb * 128:(qb + 1) * 128],
                                rhs=wo_sb[:, hp, :],
                                start=(hp == 0), stop=(hp == 1))
                        yt = ys.tile([128, HIDDEN], f32, tag="yt",
                                     name=f"yt{qb}")
                        nc.vector.tensor_copy(yt[:], yp[:])
                        nc.sync.dma_start(y_d[qb * 128:(qb + 1) * 128, :],
                                          yt[:])

    nc.compile()
    return nc


def _get_program():
    if "nc" not in _CACHE:
        _CACHE["nc"] = _build_program()
    return _CACHE["nc"]


def _prep_inputs(x, cancer_type, attn_mask, wq, bq, wk, bk, wv, bv, wo, bo,
                 bias_emb, keymod_emb):
    """Host-side shard prep: returns (in_maps list of 8, epilogue (512,))."""
    x = np.asarray(x, dtype=np.float32)
    ct = np.asarray(cancer_type).astype(np.int64)
    mask = np.asarray(attn_mask)
    wq = np.asarray(wq, dtype=np.float32)
    wk = np.asarray(wk, dtype=np.float32)
    wv = np.asarray(wv, dtype=np.float32)
    wo = np.asarray(wo, dtype=np.float32)
    bq = np.asarray(bq, dtype=np.float32)
    bk = np.asarray(bk, dtype=np.float32)
    bv = np.asarray(bv, dtype=np.float32)
    bo = np.asarray(bo, dtype=np.float32)
    keymod = np.asarray(keymod_emb, dtype=np.float32)

    wqt = np.ascontiguousarray(wq.T).astype(BF16)     # (in 512, out 512)
    wkt = np.ascontiguousarray(wk.T).astype(BF16)
    wvt = np.ascontiguousarray(wv.T).astype(BF16)
    wot = np.ascontiguousarray(wo.T).astype(BF16)

    xt_all = [np.ascontiguousarray(x[b].T).astype(BF16) for b in range(B)]
    mka = np.where(mask, np.float32(MASK_NEG), np.float32(0.0)).astype(np.float32)

    in_maps = []
    for core in range(N_CORES):
        b, g = core // 2, core % 2
        gs = slice(g * G_DIM, (g + 1) * G_DIM)
        kbias = (bk + keymod[ct[b]])[gs].reshape(2, 128, 1).astype(np.float32)
        qbias = bq[gs].reshape(2, 128, 1).astype(np.float32)
        in_maps.append({
            "xt": xt_all[b],
            "wq": np.ascontiguousarray(wqt[:, gs].reshape(4, 128, G_DIM)),
            "wk": np.ascontiguousarray(wkt[:, gs].reshape(4, 128, G_DIM)),
            "wv": np.ascontiguousarray(wvt[:, gs].reshape(4, 128, G_DIM)),
            "wo": np.ascontiguousarray(wot[gs, :].reshape(2, 128, HIDDEN)),
            "qb": qbias,
            "kb": kbias,
            "mk": np.ascontiguousarray(mka[b].reshape(NKB, 128, 1)),
        })
    epilogue = (bv @ wo.T + bo).astype(np.float32)    # (512,)
    return in_maps, epilogue


def kernel(**inputs):
    from concourse import bass_utils

    nc = _get_program()
    in_maps, epilogue = _prep_inputs(**inputs)
    res = bass_utils.run_bass_kernel_spmd(nc, in_maps,
                                          core_ids=list(range(N_CORES)))
    out = np.empty((B, N, HIDDEN), dtype=np.float32)
    for b in range(B):
        out[b] = res.results[2 * b]["y"] + res.results[2 * b + 1]["y"] + epilogue
    return out


# revision 7
# speedup vs baseline: 1.0989x; 1.0016x over previous
"""ContextConditionedAttention Trainium2 kernel.

Full-input contract: kernel(**inputs) takes the unsharded numpy inputs and
returns the full (B, N, HIDDEN) float32 output. Internally the work is
sharded over 8 NeuronCores as (batch b in 0..3) x (head-group g in 0..1),
4 heads per core. Each core computes its head-group's partial out-projection
(2048, 512); the host sums the two head-group partials per batch and adds
the bias epilogue.

Math notes (exact simplifications vs the reference):
  - per-(batch,head) softmax bias bias_emb[ct] is constant along the softmax
    axis -> cancels in softmax -> dropped.
  - keymod_emb[ct] adds to K -> folded into the K projection bias.
  - attn_mask folds into the exp() activation as a per-key additive bias
    (0 or -1e30).
  - V bias + out bias: softmax rows sum to 1 -> P@(V + 1 bv^T) = P@V + 1 bv^T,
    so host epilogue adds (bv @ wo.T + bo).

On-chip layout (per core): everything is computed transposed so no on-chip
transposes are needed:
  Q^T/K^T (d on partitions, tokens free) from lhsT=wq^T chunks, rhs=x^T;
  S^T = K @ Q^T (keys on partitions, row-tiled head pairs); P^T = exp(S^T/8
  + mask) on the scalar engine; O^T accumulated from lhsT=[V | 1] (the ones
  column yields the softmax denominator in row 64); normalization via DVE
  reciprocal + GPSIMD partition_broadcast; out-proj from lhsT=O^T blocks,
  emitted per query-chunk so the store overlaps the next chunk's attention.

PSUM (8 banks total) is a single pool: tags a0/a1 (128,1024 f32, 2 banks
each) shared by Q/K projections, score tiles, and the normalize broadcast;
tags b0/b1 (65,1024 f32, 2 banks each) shared by V projection, O^T
accumulators, and the out-projection tiles.
"""

import numpy as np
import ml_dtypes

B, N, HIDDEN = 4, 2048, 512
N_HEADS, HEAD_DIM = 8, 64
G_HEADS = 4          # heads per core (head-group)
G_DIM = 256          # dims per head-group
N_CORES = 8
NKB = N // 128       # key blocks of 128
NQB = N // 128       # query blocks of 128
QCH = 1024           # query chunk for the attention inner loop
SCALE = 1.0 / float(np.sqrt(HEAD_DIM))
MASK_NEG = -1.0e30

BF16 = ml_dtypes.bfloat16

_CACHE = {}


def _build_program():
    import concourse.bacc as bacc
    import concourse.mybir as mybir
    import concourse.tile as tile
    from concourse import bass_isa

    nc = bacc.Bacc("TRN2", target_bir_lowering=False, debug=False,
                   num_devices=N_CORES)
    f32 = mybir.dt.float32
    bf16 = mybir.dt.bfloat16
    Exp = mybir.ActivationFunctionType.Exp

    # DRAM I/O (per-core shards; same program on all 8 cores)
    xt_d = nc.dram_tensor("xt", (HIDDEN, N), bf16, kind="ExternalInput").ap()
    wq_d = nc.dram_tensor("wq", (4, 128, G_DIM), bf16, kind="ExternalInput").ap()
    wk_d = nc.dram_tensor("wk", (4, 128, G_DIM), bf16, kind="ExternalInput").ap()
    wv_d = nc.dram_tensor("wv", (4, 128, G_DIM), bf16, kind="ExternalInput").ap()
    wo_d = nc.dram_tensor("wo", (2, 128, HIDDEN), bf16, kind="ExternalInput").ap()
    qb_d = nc.dram_tensor("qb", (2, 128, 1), f32, kind="ExternalInput").ap()
    kb_d = nc.dram_tensor("kb", (2, 128, 1), f32, kind="ExternalInput").ap()
    mk_d = nc.dram_tensor("mk", (NKB, 128, 1), f32, kind="ExternalInput").ap()
    y_d = nc.dram_tensor("y", (N, HIDDEN), f32, kind="ExternalOutput").ap()

    with tile.TileContext(nc) as tc:
        with tc.tile_pool(name="sb", bufs=1) as sb, \
             tc.tile_pool(name="pp", bufs=2) as pp, \
             tc.tile_pool(name="ps", bufs=1, space="PSUM") as ps:
            # ---- persistent SBUF tiles ----
            xt_sb = [sb.tile([128, N], bf16, tag=f"xt{c}", name=f"xt{c}")
                     for c in range(4)]
            wq_sb = sb.tile([128, 4, G_DIM], bf16, tag="wq", name="wq_sb")
            wk_sb = sb.tile([128, 4, G_DIM], bf16, tag="wk", name="wk_sb")
            wv_sb = sb.tile([128, 4, G_DIM], bf16, tag="wv", name="wv_sb")
            wo_sb = sb.tile([128, 2, HIDDEN], bf16, tag="wo", name="wo_sb")
            qb_sb = sb.tile([128, 2], f32, tag="qb", name="qb_sb")
            kb_sb = sb.tile([128, 2], f32, tag="kb", name="kb_sb")
            mk_sb = sb.tile([128, NKB], f32, tag="mk", name="mk_sb")
            qt_sb = [sb.tile([128, N], bf16, tag=f"qt{hp}", name=f"qt{hp}")
                     for hp in range(2)]
            kt_sb = [sb.tile([128, N], bf16, tag=f"kt{hp}", name=f"kt{hp}")
                     for hp in range(2)]
            # V with a ones column per (key-block, head): (128, kb, h, 65)
            v_sb = sb.tile([128, NKB, G_HEADS, HEAD_DIM + 1], bf16, tag="v",
                           name="v_sb")
            ot_sb = [sb.tile([128, N], bf16, tag=f"ot{hp}", name=f"ot{hp}")
                     for hp in range(2)]

            # ---- input DMAs (need-order: weights for hp0 Q/K first) ----
            warm = sb.tile([1, 4], f32, tag="warm", name="warm")
            nc.vector.memset(warm[:], 0.0)
            nc.scalar.activation(warm[:], warm[:], Exp)  # preload Exp table
            for c in range(4):
                nc.sync.dma_start(wq_sb[:, c, :], wq_d[c])
                nc.sync.dma_start(wk_sb[:, c, :], wk_d[c])
            for hp in range(2):
                nc.sync.dma_start(qb_sb[:, hp:hp + 1], qb_d[hp])
                nc.sync.dma_start(kb_sb[:, hp:hp + 1], kb_d[hp])
            for k in range(NKB):
                nc.sync.dma_start(mk_sb[:, k:k + 1], mk_d[k])
            for c in range(4):
                nc.sync.dma_start(xt_sb[c][:], xt_d[c * 128:(c + 1) * 128, :])
            for c in range(4):
                nc.sync.dma_start(wv_sb[:, c, :], wv_d[c])
            for hp in range(2):
                nc.sync.dma_start(wo_sb[:, hp, :], wo_d[hp])
            nc.vector.memset(v_sb[:], 1.0)

            # ---- phase A: projections (psum tags shared with attention) ----
            vk = 0   # V key-block emission counter (interleaved with Q/K)

            def emit_v_block():
                nonlocal vk
                if vk >= NKB:
                    return
                k = vk
                psv = ps.tile([128, G_DIM], f32, tag=f"b{k % 2}",
                              name=f"ps_v{k}")
                for c in range(4):
                    nc.tensor.matmul(
                        psv[:],
                        lhsT=xt_sb[c][:, k * 128:(k + 1) * 128],
                        rhs=wv_sb[:, c, :],
                        start=(c == 0), stop=(c == 3))
                nc.vector.tensor_copy(
                    v_sb[:, k, :, 0:HEAD_DIM],
                    psv.rearrange("p (h d) -> p h d", h=G_HEADS))
                vk += 1

            for hp in range(2):
                for j, (w_sb, bias_sb, dst) in enumerate(
                        ((wq_sb, qb_sb, qt_sb), (wk_sb, kb_sb, kt_sb))):
                    for half in range(2):
                        pst = ps.tile([128, QCH], f32, tag=f"a{j}",
                                      name=f"ps_qk{hp}{j}{half}")
                        for c in range(4):
                            for s in range(2):
                                fr = half * QCH + s * 512
                                nc.tensor.matmul(
                                    pst[:, s * 512:(s + 1) * 512],
                                    lhsT=w_sb[:, c, hp * 128:(hp + 1) * 128],
                                    rhs=xt_sb[c][:, fr:fr + 512],
                                    start=(c == 0), stop=(c == 3))
                        nc.vector.tensor_scalar_add(
                            dst[hp][:, half * QCH:(half + 1) * QCH],
                            pst[:], bias_sb[:, hp:hp + 1])
                        if hp == 1:   # V overlaps hp1's Q/K projection
                            emit_v_block()
                            emit_v_block()
            while vk < NKB:
                emit_v_block()

            # ---- phase B: attention + per-chunk out-projection ----
            for qc in range(2):
                q0 = qc * QCH
                for hp in range(2):
                    b_ps = [ps.tile([HEAD_DIM + 1, QCH], f32, tag=f"b{i}",
                                    name=f"ps_b{qc}{hp}{i}")
                            for i in range(2)]
                    for k in range(NKB):
                        a_ps = [ps.tile([128, QCH], f32, tag=f"a{i}",
                                        name=f"ps_a{qc}{hp}{k}{i}")
                                for i in range(2)]
                        p_t = [pp.tile([128, QCH], bf16, tag=f"p{i}",
                                       name=f"p{qc}{hp}{k}{i}")
                               for i in range(2)]
                        for i in range(2):   # i = head within pair
                            r0 = i * 64
                            for s in range(2):
                                nc.tensor.matmul(
                                    a_ps[i][:, s * 512:(s + 1) * 512],
                                    lhsT=kt_sb[hp][r0:r0 + 64,
                                                   k * 128:(k + 1) * 128],
                                    rhs=qt_sb[hp][r0:r0 + 64,
                                                  q0 + s * 512:q0 + (s + 1) * 512],
                                    start=True, stop=True)
                            nc.scalar.activation(
                                p_t[i][:], a_ps[i][:], Exp,
                                bias=mk_sb[:, k:k + 1], scale=SCALE)
                            h = 2 * hp + i
                            for s in range(2):
                                nc.tensor.matmul(
                                    b_ps[i][:, s * 512:(s + 1) * 512],
                                    lhsT=v_sb[:, k, h, :],
                                    rhs=p_t[i][:, s * 512:(s + 1) * 512],
                                    start=(k == 0), stop=(k == NKB - 1))
                    # normalize: reciprocal of denominators (row 64 of b),
                    # partition_broadcast on GPSIMD, multiply into O^T sbuf
                    for i in range(2):
                        r_t = pp.tile([1, QCH], f32, tag=f"r{i}",
                                      name=f"r{qc}{hp}{i}")
                        nc.vector.reciprocal(
                            r_t[:], b_ps[i][HEAD_DIM:HEAD_DIM + 1, :])
                        rb_t = pp.tile([HEAD_DIM, QCH], f32, tag=f"rb{i}",
                                       name=f"rb{qc}{hp}{i}")
                        nc.gpsimd.partition_broadcast(rb_t[:], r_t[:])
                        nc.vector.tensor_mul(
                            ot_sb[hp][i * 64:(i + 1) * 64, q0:q0 + QCH],
                            b_ps[i][0:HEAD_DIM, :], rb_t[:])
                # out-projection for this query chunk (b-slots are free now).
                # Copies alternate DVE/ACT: both are otherwise idle here.
                with tc.tile_pool(name=f"ysb{qc}", bufs=4) as ys:
                    for j in range(NQB // 2):
                        qb = qc * (NQB // 2) + j
                        yp = ps.tile([128, HIDDEN], f32, tag=f"b{j % 2}",
                                     name=f"ps_y{qb}")
                        for hp in range(2):
                            nc.tensor.matmul(
                                yp[:],
                                lhsT=ot_sb[hp][:, qb * 128:(qb + 1) * 128],
                                rhs=wo_sb[:, hp, :],
                                start=(hp == 0), stop=(hp == 1))
                        yt = ys.tile([128, HIDDEN], f32, tag="yt",
                                     name=f"yt{qb}")
                        if j % 2 == 0:
                            nc.vector.tensor_copy(yt[:], yp[:])
                        else:
                            nc.scalar.copy(yt[:], yp[:])
                        nc.sync.dma_start(y_d[qb * 128:(qb + 1) * 128, :],
                                          yt[:])

    nc.compile()
    return nc


def _get_program():
    if "nc" not in _CACHE:
        _CACHE["nc"] = _build_program()
    return _CACHE["nc"]


def _prep_inputs(x, cancer_type, attn_mask, wq, bq, wk, bk, wv, bv, wo, bo,
                 bias_emb, keymod_emb):
    """Host-side shard prep: returns (in_maps list of 8, epilogue (512,))."""
    x = np.asarray(x, dtype=np.float32)
    ct = np.asarray(cancer_type).astype(np.int64)
    mask = np.asarray(attn_mask)
    wq = np.asarray(wq, dtype=np.float32)
    wk = np.asarray(wk, dtype=np.float32)
    wv = np.asarray(wv, dtype=np.float32)
    wo = np.asarray(wo, dtype=np.float32)
    bq = np.asarray(bq, dtype=np.float32)
    bk = np.asarray(bk, dtype=np.float32)
    bv = np.asarray(bv, dtype=np.float32)
    bo = np.asarray(bo, dtype=np.float32)
    keymod = np.asarray(keymod_emb, dtype=np.float32)

    wqt = np.ascontiguousarray(wq.T).astype(BF16)     # (in 512, out 512)
    wkt = np.ascontiguousarray(wk.T).astype(BF16)
    wvt = np.ascontiguousarray(wv.T).astype(BF16)
    wot = np.ascontiguousarray(wo.T).astype(BF16)

    xt_all = [np.ascontiguousarray(x[b].T).astype(BF16) for b in range(B)]
    mka = np.where(mask, np.float32(MASK_NEG), np.float32(0.0)).astype(np.float32)

    in_maps = []
    for core in range(N_CORES):
        b, g = core // 2, core % 2
        gs = slice(g * G_DIM, (g + 1) * G_DIM)
        kbias = (bk + keymod[ct[b]])[gs].reshape(2, 128, 1).astype(np.float32)
        qbias = bq[gs].reshape(2, 128, 1).astype(np.float32)
        in_maps.append({
            "xt": xt_all[b],
            "wq": np.ascontiguousarray(wqt[:, gs].reshape(4, 128, G_DIM)),
            "wk": np.ascontiguousarray(wkt[:, gs].reshape(4, 128, G_DIM)),
            "wv": np.ascontiguousarray(wvt[:, gs].reshape(4, 128, G_DIM)),
            "wo": np.ascontiguousarray(wot[gs, :].reshape(2, 128, HIDDEN)),
            "qb": qbias,
            "kb": kbias,
            "mk": np.ascontiguousarray(mka[b].reshape(NKB, 128, 1)),
        })
    epilogue = (bv @ wo.T + bo).astype(np.float32)    # (512,)
    return in_maps, epilogue


def kernel(**inputs):
    from concourse import bass_utils

    nc = _get_program()
    in_maps, epilogue = _prep_inputs(**inputs)
    res = bass_utils.run_bass_kernel_spmd(nc, in_maps,
                                          core_ids=list(range(N_CORES)))
    out = np.empty((B, N, HIDDEN), dtype=np.float32)
    for b in range(B):
        out[b] = res.results[2 * b]["y"] + res.results[2 * b + 1]["y"] + epilogue
    return out


# revision 8
# speedup vs baseline: 1.1574x; 1.0532x over previous
"""ContextConditionedAttention Trainium2 kernel.

Full-input contract: kernel(**inputs) takes the unsharded numpy inputs and
returns the full (B, N, HIDDEN) float32 output. Internally the work is
sharded over 8 NeuronCores as (batch b in 0..3) x (head-group g in 0..1),
4 heads per core. Each core computes its head-group's partial out-projection
(2048, 512); the host sums the two head-group partials per batch and adds
the bias epilogue.

Math notes (exact simplifications vs the reference):
  - per-(batch,head) softmax bias bias_emb[ct] is constant along the softmax
    axis -> cancels in softmax -> dropped.
  - keymod_emb[ct] adds to K -> folded into the K projection bias.
  - attn_mask folds into the exp() activation as a per-key additive bias
    (0 or -1e30).
  - V bias + out bias: softmax rows sum to 1 -> P@(V + 1 bv^T) = P@V + 1 bv^T,
    so host epilogue adds (bv @ wo.T + bo).

On-chip layout (per core): everything is computed transposed so no on-chip
transposes are needed:
  Q^T/K^T (d on partitions, tokens free) from lhsT=wq^T chunks, rhs=x^T;
  S^T = K @ Q^T (keys on partitions, row-tiled head pairs); P^T = exp(S^T/8
  + mask) on the scalar engine; O^T accumulated from lhsT=[V | 1] (the ones
  column yields the softmax denominator in row 64); normalization via DVE
  reciprocal + GPSIMD partition_broadcast; out-proj from lhsT=O^T blocks,
  emitted per query-chunk so the store overlaps the next chunk's attention.

PSUM (8 banks total) is a single pool: tags a0/a1 (128,1024 f32, 2 banks
each) shared by Q/K projections, score tiles, and the normalize broadcast;
tags b0/b1 (65,1024 f32, 2 banks each) shared by V projection, O^T
accumulators, and the out-projection tiles.
"""

import numpy as np
import ml_dtypes

B, N, HIDDEN = 4, 2048, 512
N_HEADS, HEAD_DIM = 8, 64
G_HEADS = 4          # heads per core (head-group)
G_DIM = 256          # dims per head-group
N_CORES = 8
NKB = N // 128       # key blocks of 128
NQB = N // 128       # query blocks of 128
QCH = 1024           # query chunk for the attention inner loop
SCALE = 1.0 / float(np.sqrt(HEAD_DIM))
MASK_NEG = -1.0e30

BF16 = ml_dtypes.bfloat16

_CACHE = {}


def _build_program():
    import concourse.bacc as bacc
    import concourse.mybir as mybir
    import concourse.tile as tile
    from concourse import bass_isa

    nc = bacc.Bacc("TRN2", target_bir_lowering=False, debug=False,
                   num_devices=N_CORES)
    f32 = mybir.dt.float32
    bf16 = mybir.dt.bfloat16
    Exp = mybir.ActivationFunctionType.Exp

    # DRAM I/O (per-core shards; same program on all 8 cores)
    xt_d = nc.dram_tensor("xt", (HIDDEN, N), bf16, kind="ExternalInput").ap()
    wq_d = nc.dram_tensor("wq", (4, 128, G_DIM), bf16, kind="ExternalInput").ap()
    wk_d = nc.dram_tensor("wk", (4, 128, G_DIM), bf16, kind="ExternalInput").ap()
    wv_d = nc.dram_tensor("wv", (4, 128, G_DIM), bf16, kind="ExternalInput").ap()
    wo_d = nc.dram_tensor("wo", (2, 128, HIDDEN), bf16, kind="ExternalInput").ap()
    qb_d = nc.dram_tensor("qb", (128, 2), f32, kind="ExternalInput").ap()
    kb_d = nc.dram_tensor("kb", (128, 2), f32, kind="ExternalInput").ap()
    mk_d = nc.dram_tensor("mk", (128, NKB), f32, kind="ExternalInput").ap()
    y_d = nc.dram_tensor("y", (N, HIDDEN), f32, kind="ExternalOutput").ap()

    with tile.TileContext(nc) as tc:
        with tc.tile_pool(name="sb", bufs=1) as sb, \
             tc.tile_pool(name="pp", bufs=2) as pp, \
             tc.tile_pool(name="ps", bufs=1, space="PSUM") as ps:
            # ---- persistent SBUF tiles ----
            xt_sb = [sb.tile([128, N], bf16, tag=f"xt{c}", name=f"xt{c}")
                     for c in range(4)]
            wq_sb = sb.tile([128, 4, G_DIM], bf16, tag="wq", name="wq_sb")
            wk_sb = sb.tile([128, 4, G_DIM], bf16, tag="wk", name="wk_sb")
            wv_sb = sb.tile([128, 4, G_DIM], bf16, tag="wv", name="wv_sb")
            wo_sb = sb.tile([128, 2, HIDDEN], bf16, tag="wo", name="wo_sb")
            qb_sb = sb.tile([128, 2], f32, tag="qb", name="qb_sb")
            kb_sb = sb.tile([128, 2], f32, tag="kb", name="kb_sb")
            mk_sb = sb.tile([128, NKB], f32, tag="mk", name="mk_sb")
            qt_sb = [sb.tile([128, N], bf16, tag=f"qt{hp}", name=f"qt{hp}")
                     for hp in range(2)]
            kt_sb = [sb.tile([128, N], bf16, tag=f"kt{hp}", name=f"kt{hp}")
                     for hp in range(2)]
            # V with a ones column per (key-block, head): (128, kb, h, 65)
            v_sb = sb.tile([128, NKB, G_HEADS, HEAD_DIM + 1], bf16, tag="v",
                           name="v_sb")
            ot_sb = [sb.tile([128, N], bf16, tag=f"ot{hp}", name=f"ot{hp}")
                     for hp in range(2)]

            # ---- input DMAs (need-order: weights for hp0 Q/K first) ----
            warm = sb.tile([1, 4], f32, tag="warm", name="warm")
            nc.vector.memset(warm[:], 0.0)
            nc.scalar.activation(warm[:], warm[:], Exp)  # preload Exp table
            for c in range(4):
                nc.sync.dma_start(wq_sb[:, c, :], wq_d[c])
                nc.sync.dma_start(wk_sb[:, c, :], wk_d[c])
            nc.sync.dma_start(qb_sb[:], qb_d[:])
            nc.sync.dma_start(kb_sb[:], kb_d[:])
            nc.sync.dma_start(mk_sb[:], mk_d[:])
            for c in range(4):
                nc.sync.dma_start(xt_sb[c][:], xt_d[c * 128:(c + 1) * 128, :])
            for c in range(4):
                nc.sync.dma_start(wv_sb[:, c, :], wv_d[c])
            for hp in range(2):
                nc.sync.dma_start(wo_sb[:, hp, :], wo_d[hp])
            nc.vector.memset(v_sb[:], 1.0)

            # ---- phase A: projections (psum tags shared with attention) ----
            vk = 0   # V key-block emission counter (interleaved with Q/K)

            def emit_v_block():
                nonlocal vk
                if vk >= NKB:
                    return
                k = vk
                psv = ps.tile([128, G_DIM], f32, tag=f"b{k % 2}",
                              name=f"ps_v{k}")
                for c in range(4):
                    nc.tensor.matmul(
                        psv[:],
                        lhsT=xt_sb[c][:, k * 128:(k + 1) * 128],
                        rhs=wv_sb[:, c, :],
                        start=(c == 0), stop=(c == 3))
                nc.vector.tensor_copy(
                    v_sb[:, k, :, 0:HEAD_DIM],
                    psv.rearrange("p (h d) -> p h d", h=G_HEADS))
                vk += 1

            for hp in range(2):
                for j, (w_sb, bias_sb, dst) in enumerate(
                        ((wq_sb, qb_sb, qt_sb), (wk_sb, kb_sb, kt_sb))):
                    for half in range(2):
                        pst = ps.tile([128, QCH], f32, tag=f"a{j}",
                                      name=f"ps_qk{hp}{j}{half}")
                        for c in range(4):
                            for s in range(2):
                                fr = half * QCH + s * 512
                                nc.tensor.matmul(
                                    pst[:, s * 512:(s + 1) * 512],
                                    lhsT=w_sb[:, c, hp * 128:(hp + 1) * 128],
                                    rhs=xt_sb[c][:, fr:fr + 512],
                                    start=(c == 0), stop=(c == 3))
                        nc.vector.tensor_scalar_add(
                            dst[hp][:, half * QCH:(half + 1) * QCH],
                            pst[:], bias_sb[:, hp:hp + 1])
                        if hp == 1:   # V overlaps hp1's Q/K projection
                            emit_v_block()
                            emit_v_block()
            while vk < NKB:
                emit_v_block()

            # ---- phase B: attention + per-chunk out-projection ----
            for qc in range(2):
                q0 = qc * QCH
                for hp in range(2):
                    b_ps = [ps.tile([HEAD_DIM + 1, QCH], f32, tag=f"b{i}",
                                    name=f"ps_b{qc}{hp}{i}")
                            for i in range(2)]
                    for k in range(NKB):
                        a_ps = [ps.tile([128, QCH], f32, tag=f"a{i}",
                                        name=f"ps_a{qc}{hp}{k}{i}")
                                for i in range(2)]
                        p_t = [pp.tile([128, QCH], bf16, tag=f"p{i}",
                                       name=f"p{qc}{hp}{k}{i}")
                               for i in range(2)]
                        for i in range(2):   # i = head within pair
                            r0 = i * 64
                            for s in range(2):
                                nc.tensor.matmul(
                                    a_ps[i][:, s * 512:(s + 1) * 512],
                                    lhsT=kt_sb[hp][r0:r0 + 64,
                                                   k * 128:(k + 1) * 128],
                                    rhs=qt_sb[hp][r0:r0 + 64,
                                                  q0 + s * 512:q0 + (s + 1) * 512],
                                    start=True, stop=True)
                            nc.scalar.activation(
                                p_t[i][:], a_ps[i][:], Exp,
                                bias=mk_sb[:, k:k + 1], scale=SCALE)
                            h = 2 * hp + i
                            for s in range(2):
                                nc.tensor.matmul(
                                    b_ps[i][:, s * 512:(s + 1) * 512],
                                    lhsT=v_sb[:, k, h, :],
                                    rhs=p_t[i][:, s * 512:(s + 1) * 512],
                                    start=(k == 0), stop=(k == NKB - 1))
                    # normalize: reciprocal of denominators (row 64 of b),
                    # partition_broadcast on GPSIMD, multiply into O^T sbuf
                    for i in range(2):
                        r_t = pp.tile([1, QCH], f32, tag=f"r{i}",
                                      name=f"r{qc}{hp}{i}")
                        nc.vector.reciprocal(
                            r_t[:], b_ps[i][HEAD_DIM:HEAD_DIM + 1, :])
                        rb_t = pp.tile([HEAD_DIM, QCH], f32, tag=f"rb{i}",
                                       name=f"rb{qc}{hp}{i}")
                        nc.gpsimd.partition_broadcast(rb_t[:], r_t[:])
                        nc.vector.tensor_mul(
                            ot_sb[hp][i * 64:(i + 1) * 64, q0:q0 + QCH],
                            b_ps[i][0:HEAD_DIM, :], rb_t[:])
                # out-projection for this query chunk (b-slots are free now).
                # Copies alternate DVE/ACT: both are otherwise idle here.
                with tc.tile_pool(name=f"ysb{qc}", bufs=4) as ys:
                    for j in range(NQB // 2):
                        qb = qc * (NQB // 2) + j
                        yp = ps.tile([128, HIDDEN], f32, tag=f"b{j % 2}",
                                     name=f"ps_y{qb}")
                        for hp in range(2):
                            nc.tensor.matmul(
                                yp[:],
                                lhsT=ot_sb[hp][:, qb * 128:(qb + 1) * 128],
                                rhs=wo_sb[:, hp, :],
                                start=(hp == 0), stop=(hp == 1))
                        yt = ys.tile([128, HIDDEN], f32, tag="yt",
                                     name=f"yt{qb}")
                        if j % 2 == 0:
                            nc.vector.tensor_copy(yt[:], yp[:])
                        else:
                            nc.scalar.copy(yt[:], yp[:])
                        nc.sync.dma_start(y_d[qb * 128:(qb + 1) * 128, :],
                                          yt[:])

    nc.compile()
    return nc


def _get_program():
    if "nc" not in _CACHE:
        _CACHE["nc"] = _build_program()
    return _CACHE["nc"]


def _prep_inputs(x, cancer_type, attn_mask, wq, bq, wk, bk, wv, bv, wo, bo,
                 bias_emb, keymod_emb):
    """Host-side shard prep: returns (in_maps list of 8, epilogue (512,))."""
    x = np.asarray(x, dtype=np.float32)
    ct = np.asarray(cancer_type).astype(np.int64)
    mask = np.asarray(attn_mask)
    wq = np.asarray(wq, dtype=np.float32)
    wk = np.asarray(wk, dtype=np.float32)
    wv = np.asarray(wv, dtype=np.float32)
    wo = np.asarray(wo, dtype=np.float32)
    bq = np.asarray(bq, dtype=np.float32)
    bk = np.asarray(bk, dtype=np.float32)
    bv = np.asarray(bv, dtype=np.float32)
    bo = np.asarray(bo, dtype=np.float32)
    keymod = np.asarray(keymod_emb, dtype=np.float32)

    wqt = np.ascontiguousarray(wq.T).astype(BF16)     # (in 512, out 512)
    wkt = np.ascontiguousarray(wk.T).astype(BF16)
    wvt = np.ascontiguousarray(wv.T).astype(BF16)
    wot = np.ascontiguousarray(wo.T).astype(BF16)

    xt_all = [np.ascontiguousarray(x[b].T).astype(BF16) for b in range(B)]
    mka = np.where(mask, np.float32(MASK_NEG), np.float32(0.0)).astype(np.float32)

    in_maps = []
    for core in range(N_CORES):
        b, g = core // 2, core % 2
        gs = slice(g * G_DIM, (g + 1) * G_DIM)
        kbias = np.ascontiguousarray(
            (bk + keymod[ct[b]])[gs].reshape(2, 128).T).astype(np.float32)
        qbias = np.ascontiguousarray(bq[gs].reshape(2, 128).T).astype(np.float32)
        in_maps.append({
            "xt": xt_all[b],
            "wq": np.ascontiguousarray(wqt[:, gs].reshape(4, 128, G_DIM)),
            "wk": np.ascontiguousarray(wkt[:, gs].reshape(4, 128, G_DIM)),
            "wv": np.ascontiguousarray(wvt[:, gs].reshape(4, 128, G_DIM)),
            "wo": np.ascontiguousarray(wot[gs, :].reshape(2, 128, HIDDEN)),
            "qb": qbias,
            "kb": kbias,
            "mk": np.ascontiguousarray(mka[b].reshape(NKB, 128).T),
        })
    epilogue = (bv @ wo.T + bo).astype(np.float32)    # (512,)
    return in_maps, epilogue


def kernel(**inputs):
    from concourse import bass_utils

    nc = _get_program()
    in_maps, epilogue = _prep_inputs(**inputs)
    res = bass_utils.run_bass_kernel_spmd(nc, in_maps,
                                          core_ids=list(range(N_CORES)))
    out = np.empty((B, N, HIDDEN), dtype=np.float32)
    for b in range(B):
        out[b] = res.results[2 * b]["y"] + res.results[2 * b + 1]["y"] + epilogue
    return out


# revision 9
# speedup vs baseline: 1.2028x; 1.0392x over previous
"""ContextConditionedAttention Trainium2 kernel.

Full-input contract: kernel(**inputs) takes the unsharded numpy inputs and
returns the full (B, N, HIDDEN) float32 output. Internally the work is
sharded over 8 NeuronCores as (batch b in 0..3) x (head-group g in 0..1),
4 heads per core. Each core computes its head-group's partial out-projection
(2048, 512); the host sums the two head-group partials per batch and adds
the bias epilogue.

Math notes (exact simplifications vs the reference):
  - per-(batch,head) softmax bias bias_emb[ct] is constant along the softmax
    axis -> cancels in softmax -> dropped.
  - keymod_emb[ct] adds to K -> folded into the K projection bias.
  - attn_mask folds into the exp() activation as a per-key additive bias
    (0 or -1e30).
  - V bias + out bias: softmax rows sum to 1 -> P@(V + 1 bv^T) = P@V + 1 bv^T,
    so host epilogue adds (bv @ wo.T + bo).

On-chip layout (per core): everything is computed transposed so no on-chip
transposes are needed:
  Q^T/K^T (d on partitions, tokens free) from lhsT=wq^T chunks, rhs=x^T;
  S^T = K @ Q^T (keys on partitions, row-tiled head pairs); P^T = exp(S^T/8
  + mask) on the scalar engine; O^T accumulated from lhsT=[V | 1] (the ones
  column yields the softmax denominator in row 64); normalization via DVE
  reciprocal + GPSIMD partition_broadcast; out-proj from lhsT=O^T blocks,
  emitted per query-chunk so the store overlaps the next chunk's attention.

PSUM (8 banks total) is a single pool: tags a0/a1 (128,1024 f32, 2 banks
each) shared by Q/K projections, score tiles, and the normalize broadcast;
tags b0/b1 (65,1024 f32, 2 banks each) shared by V projection, O^T
accumulators, and the out-projection tiles.
"""

import numpy as np
import ml_dtypes

B, N, HIDDEN = 4, 2048, 512
N_HEADS, HEAD_DIM = 8, 64
G_HEADS = 4          # heads per core (head-group)
G_DIM = 256          # dims per head-group
N_CORES = 8
NKB = N // 128       # key blocks of 128
NQB = N // 128       # query blocks of 128
QCH = 1024           # query chunk for the attention inner loop
SCALE = 1.0 / float(np.sqrt(HEAD_DIM))
MASK_NEG = -1.0e30

BF16 = ml_dtypes.bfloat16

_CACHE = {}


def _build_program():
    import concourse.bacc as bacc
    import concourse.mybir as mybir
    import concourse.tile as tile
    from concourse import bass_isa

    nc = bacc.Bacc("TRN2", target_bir_lowering=False, debug=False,
                   num_devices=N_CORES)
    f32 = mybir.dt.float32
    bf16 = mybir.dt.bfloat16
    Exp = mybir.ActivationFunctionType.Exp

    # DRAM I/O (per-core shards; same program on all 8 cores)
    xt_d = nc.dram_tensor("xt", (HIDDEN, N), bf16, kind="ExternalInput").ap()
    wq_d = nc.dram_tensor("wq", (128, 4, G_DIM), bf16, kind="ExternalInput").ap()
    wk_d = nc.dram_tensor("wk", (128, 4, G_DIM), bf16, kind="ExternalInput").ap()
    wv_d = nc.dram_tensor("wv", (128, 4, G_DIM), bf16, kind="ExternalInput").ap()
    wo_d = nc.dram_tensor("wo", (128, 2, HIDDEN), bf16, kind="ExternalInput").ap()
    qb_d = nc.dram_tensor("qb", (128, 2), f32, kind="ExternalInput").ap()
    kb_d = nc.dram_tensor("kb", (128, 2), f32, kind="ExternalInput").ap()
    mk_d = nc.dram_tensor("mk", (128, NKB), f32, kind="ExternalInput").ap()
    y_d = nc.dram_tensor("y", (N, HIDDEN), f32, kind="ExternalOutput").ap()

    with tile.TileContext(nc) as tc:
        with tc.tile_pool(name="sb", bufs=1) as sb, \
             tc.tile_pool(name="pp", bufs=2) as pp, \
             tc.tile_pool(name="ps", bufs=1, space="PSUM") as ps:
            # ---- persistent SBUF tiles ----
            xt_sb = [sb.tile([128, N], bf16, tag=f"xt{c}", name=f"xt{c}")
                     for c in range(4)]
            wq_sb = sb.tile([128, 4, G_DIM], bf16, tag="wq", name="wq_sb")
            wk_sb = sb.tile([128, 4, G_DIM], bf16, tag="wk", name="wk_sb")
            wv_sb = sb.tile([128, 4, G_DIM], bf16, tag="wv", name="wv_sb")
            wo_sb = sb.tile([128, 2, HIDDEN], bf16, tag="wo", name="wo_sb")
            qb_sb = sb.tile([128, 2], f32, tag="qb", name="qb_sb")
            kb_sb = sb.tile([128, 2], f32, tag="kb", name="kb_sb")
            mk_sb = sb.tile([128, NKB], f32, tag="mk", name="mk_sb")
            qt_sb = [sb.tile([128, N], bf16, tag=f"qt{hp}", name=f"qt{hp}")
                     for hp in range(2)]
            kt_sb = [sb.tile([128, N], bf16, tag=f"kt{hp}", name=f"kt{hp}")
                     for hp in range(2)]
            # V with a ones column per (key-block, head): (128, kb, h, 65)
            v_sb = sb.tile([128, NKB, G_HEADS, HEAD_DIM + 1], bf16, tag="v",
                           name="v_sb")
            ot_sb = [sb.tile([128, N], bf16, tag=f"ot{hp}", name=f"ot{hp}")
                     for hp in range(2)]

            # ---- input DMAs (need-order: weights for hp0 Q/K first) ----
            warm = sb.tile([1, 4], f32, tag="warm", name="warm")
            nc.vector.memset(warm[:], 0.0)
            nc.scalar.activation(warm[:], warm[:], Exp)  # preload Exp table
            nc.sync.dma_start(wq_sb[:], wq_d[:])
            nc.sync.dma_start(wk_sb[:], wk_d[:])
            for c in range(4):
                nc.sync.dma_start(xt_sb[c][:], xt_d[c * 128:(c + 1) * 128, :])
            nc.gpsimd.dma_start(qb_sb[:], qb_d[:])
            nc.gpsimd.dma_start(kb_sb[:], kb_d[:])
            nc.gpsimd.dma_start(mk_sb[:], mk_d[:])
            nc.gpsimd.dma_start(wv_sb[:], wv_d[:])
            nc.gpsimd.dma_start(wo_sb[:], wo_d[:])
            nc.vector.memset(v_sb[:], 1.0)

            # ---- phase A: projections (psum tags shared with attention) ----
            vk = 0   # V key-block emission counter (interleaved with Q/K)

            def emit_v_block():
                nonlocal vk
                if vk >= NKB:
                    return
                k = vk
                psv = ps.tile([128, G_DIM], f32, tag=f"b{k % 2}",
                              name=f"ps_v{k}")
                for c in range(4):
                    nc.tensor.matmul(
                        psv[:],
                        lhsT=xt_sb[c][:, k * 128:(k + 1) * 128],
                        rhs=wv_sb[:, c, :],
                        start=(c == 0), stop=(c == 3))
                nc.vector.tensor_copy(
                    v_sb[:, k, :, 0:HEAD_DIM],
                    psv.rearrange("p (h d) -> p h d", h=G_HEADS))
                vk += 1

            for hp in range(2):
                for j, (w_sb, bias_sb, dst) in enumerate(
                        ((wq_sb, qb_sb, qt_sb), (wk_sb, kb_sb, kt_sb))):
                    for half in range(2):
                        pst = ps.tile([128, QCH], f32, tag=f"a{j}",
                                      name=f"ps_qk{hp}{j}{half}")
                        for c in range(4):
                            for s in range(2):
                                fr = half * QCH + s * 512
                                nc.tensor.matmul(
                                    pst[:, s * 512:(s + 1) * 512],
                                    lhsT=w_sb[:, c, hp * 128:(hp + 1) * 128],
                                    rhs=xt_sb[c][:, fr:fr + 512],
                                    start=(c == 0), stop=(c == 3))
                        nc.vector.tensor_scalar_add(
                            dst[hp][:, half * QCH:(half + 1) * QCH],
                            pst[:], bias_sb[:, hp:hp + 1])
                        if hp == 1:   # V overlaps hp1's Q/K projection
                            emit_v_block()
                            emit_v_block()
            while vk < NKB:
                emit_v_block()

            # ---- phase B: attention + per-chunk out-projection ----
            for qc in range(2):
                q0 = qc * QCH
                for hp in range(2):
                    b_ps = [ps.tile([HEAD_DIM + 1, QCH], f32, tag=f"b{i}",
                                    name=f"ps_b{qc}{hp}{i}")
                            for i in range(2)]
                    for k in range(NKB):
                        a_ps = [ps.tile([128, QCH], f32, tag=f"a{i}",
                                        name=f"ps_a{qc}{hp}{k}{i}")
                                for i in range(2)]
                        p_t = [pp.tile([128, QCH], bf16, tag=f"p{i}",
                                       name=f"p{qc}{hp}{k}{i}")
                               for i in range(2)]
                        for i in range(2):   # i = head within pair
                            r0 = i * 64
                            for s in range(2):
                                nc.tensor.matmul(
                                    a_ps[i][:, s * 512:(s + 1) * 512],
                                    lhsT=kt_sb[hp][r0:r0 + 64,
                                                   k * 128:(k + 1) * 128],
                                    rhs=qt_sb[hp][r0:r0 + 64,
                                                  q0 + s * 512:q0 + (s + 1) * 512],
                                    start=True, stop=True)
                            nc.scalar.activation(
                                p_t[i][:], a_ps[i][:], Exp,
                                bias=mk_sb[:, k:k + 1], scale=SCALE)
                            h = 2 * hp + i
                            for s in range(2):
                                nc.tensor.matmul(
                                    b_ps[i][:, s * 512:(s + 1) * 512],
                                    lhsT=v_sb[:, k, h, :],
                                    rhs=p_t[i][:, s * 512:(s + 1) * 512],
                                    start=(k == 0), stop=(k == NKB - 1))
                    # normalize: reciprocal of denominators (row 64 of b),
                    # partition_broadcast on GPSIMD, multiply into O^T sbuf
                    for i in range(2):
                        r_t = pp.tile([1, QCH], f32, tag=f"r{i}",
                                      name=f"r{qc}{hp}{i}")
                        nc.vector.reciprocal(
                            r_t[:], b_ps[i][HEAD_DIM:HEAD_DIM + 1, :])
                        rb_t = pp.tile([HEAD_DIM, QCH], f32, tag=f"rb{i}",
                                       name=f"rb{qc}{hp}{i}")
                        nc.gpsimd.partition_broadcast(rb_t[:], r_t[:])
                        nc.vector.tensor_mul(
                            ot_sb[hp][i * 64:(i + 1) * 64, q0:q0 + QCH],
                            b_ps[i][0:HEAD_DIM, :], rb_t[:])
                # out-projection for this query chunk (b-slots are free now).
                # Copies alternate DVE/ACT: both are otherwise idle here.
                with tc.tile_pool(name=f"ysb{qc}", bufs=4) as ys:
                    for j in range(NQB // 2):
                        qb = qc * (NQB // 2) + j
                        yp = ps.tile([128, HIDDEN], f32, tag=f"b{j % 2}",
                                     name=f"ps_y{qb}")
                        for hp in range(2):
                            nc.tensor.matmul(
                                yp[:],
                                lhsT=ot_sb[hp][:, qb * 128:(qb + 1) * 128],
                                rhs=wo_sb[:, hp, :],
                                start=(hp == 0), stop=(hp == 1))
                        yt = ys.tile([128, HIDDEN], f32, tag="yt",
                                     name=f"yt{qb}")
                        if j % 2 == 0:
                            nc.vector.tensor_copy(yt[:], yp[:])
                        else:
                            nc.scalar.copy(yt[:], yp[:])
                        nc.sync.dma_start(y_d[qb * 128:(qb + 1) * 128, :],
                                          yt[:])

    nc.compile()
    return nc


def _get_program():
    if "nc" not in _CACHE:
        _CACHE["nc"] = _build_program()
    return _CACHE["nc"]


def _prep_inputs(x, cancer_type, attn_mask, wq, bq, wk, bk, wv, bv, wo, bo,
                 bias_emb, keymod_emb):
    """Host-side shard prep: returns (in_maps list of 8, epilogue (512,))."""
    x = np.asarray(x, dtype=np.float32)
    ct = np.asarray(cancer_type).astype(np.int64)
    mask = np.asarray(attn_mask)
    wq = np.asarray(wq, dtype=np.float32)
    wk = np.asarray(wk, dtype=np.float32)
    wv = np.asarray(wv, dtype=np.float32)
    wo = np.asarray(wo, dtype=np.float32)
    bq = np.asarray(bq, dtype=np.float32)
    bk = np.asarray(bk, dtype=np.float32)
    bv = np.asarray(bv, dtype=np.float32)
    bo = np.asarray(bo, dtype=np.float32)
    keymod = np.asarray(keymod_emb, dtype=np.float32)

    wqt = np.ascontiguousarray(wq.T).astype(BF16)     # (in 512, out 512)
    wkt = np.ascontiguousarray(wk.T).astype(BF16)
    wvt = np.ascontiguousarray(wv.T).astype(BF16)
    wot = np.ascontiguousarray(wo.T).astype(BF16)

    xt_all = [np.ascontiguousarray(x[b].T).astype(BF16) for b in range(B)]
    mka = np.where(mask, np.float32(MASK_NEG), np.float32(0.0)).astype(np.float32)

    in_maps = []
    for core in range(N_CORES):
        b, g = core // 2, core % 2
        gs = slice(g * G_DIM, (g + 1) * G_DIM)
        kbias = np.ascontiguousarray(
            (bk + keymod[ct[b]])[gs].reshape(2, 128).T).astype(np.float32)
        qbias = np.ascontiguousarray(bq[gs].reshape(2, 128).T).astype(np.float32)
        in_maps.append({
            "xt": xt_all[b],
            "wq": np.ascontiguousarray(
                wqt[:, gs].reshape(4, 128, G_DIM).transpose(1, 0, 2)),
            "wk": np.ascontiguousarray(
                wkt[:, gs].reshape(4, 128, G_DIM).transpose(1, 0, 2)),
            "wv": np.ascontiguousarray(
                wvt[:, gs].reshape(4, 128, G_DIM).transpose(1, 0, 2)),
            "wo": np.ascontiguousarray(
                wot[gs, :].reshape(2, 128, HIDDEN).transpose(1, 0, 2)),
            "qb": qbias,
            "kb": kbias,
            "mk": np.ascontiguousarray(mka[b].reshape(NKB, 128).T),
        })
    epilogue = (bv @ wo.T + bo).astype(np.float32)    # (512,)
    return in_maps, epilogue


def kernel(**inputs):
    from concourse import bass_utils

    nc = _get_program()
    in_maps, epilogue = _prep_inputs(**inputs)
    res = bass_utils.run_bass_kernel_spmd(nc, in_maps,
                                          core_ids=list(range(N_CORES)))
    out = np.empty((B, N, HIDDEN), dtype=np.float32)
    for b in range(B):
        out[b] = res.results[2 * b]["y"] + res.results[2 * b + 1]["y"] + epilogue
    return out


# revision 10
# speedup vs baseline: 1.2177x; 1.0124x over previous
"""ContextConditionedAttention Trainium2 kernel.

Full-input contract: kernel(**inputs) takes the unsharded numpy inputs and
returns the full (B, N, HIDDEN) float32 output. Internally the work is
sharded over 8 NeuronCores as (batch b in 0..3) x (head-group g in 0..1),
4 heads per core. Each core computes its head-group's partial out-projection
(2048, 512); the host sums the two head-group partials per batch and adds
the bias epilogue.

Math notes (exact simplifications vs the reference):
  - per-(batch,head) softmax bias bias_emb[ct] is constant along the softmax
    axis -> cancels in softmax -> dropped.
  - keymod_emb[ct] adds to K -> folded into the K projection bias.
  - attn_mask folds into the exp() activation as a per-key additive bias
    (0 or -1e30).
  - V bias + out bias: softmax rows sum to 1 -> P@(V + 1 bv^T) = P@V + 1 bv^T,
    so host epilogue adds (bv @ wo.T + bo).

On-chip layout (per core): everything is computed transposed so no on-chip
transposes are needed:
  Q^T/K^T (d on partitions, tokens free) from lhsT=wq^T chunks, rhs=x^T;
  S^T = K @ Q^T (keys on partitions, row-tiled head pairs); P^T = exp(S^T/8
  + mask) on the scalar engine; O^T accumulated from lhsT=[V | 1] (the ones
  column yields the softmax denominator in row 64); normalization via DVE
  reciprocal + GPSIMD partition_broadcast; out-proj from lhsT=O^T blocks,
  emitted per query-chunk so the store overlaps the next chunk's attention.

PSUM (8 banks total) is a single pool: tags a0/a1 (128,1024 f32, 2 banks
each) shared by Q/K projections, score tiles, and the normalize broadcast;
tags b0/b1 (65,1024 f32, 2 banks each) shared by V projection, O^T
accumulators, and the out-projection tiles.
"""

import numpy as np
import ml_dtypes

B, N, HIDDEN = 4, 2048, 512
N_HEADS, HEAD_DIM = 8, 64
G_HEADS = 4          # heads per core (head-group)
G_DIM = 256          # dims per head-group
N_CORES = 8
NKB = N // 128       # key blocks of 128
NQB = N // 128       # query blocks of 128
QCH = 1024           # query chunk for the attention inner loop
SCALE = 1.0 / float(np.sqrt(HEAD_DIM))
MASK_NEG = -1.0e30

BF16 = ml_dtypes.bfloat16

_CACHE = {}


def _build_program():
    import concourse.bacc as bacc
    import concourse.mybir as mybir
    import concourse.tile as tile
    from concourse import bass_isa

    nc = bacc.Bacc("TRN2", target_bir_lowering=False, debug=False,
                   num_devices=N_CORES)
    f32 = mybir.dt.float32
    bf16 = mybir.dt.bfloat16
    Exp = mybir.ActivationFunctionType.Exp

    # DRAM I/O (per-core shards; same program on all 8 cores)
    xt_d = nc.dram_tensor("xt", (HIDDEN, N), bf16, kind="ExternalInput").ap()
    wq_d = nc.dram_tensor("wq", (128, 4, G_DIM), bf16, kind="ExternalInput").ap()
    wk_d = nc.dram_tensor("wk", (128, 4, G_DIM), bf16, kind="ExternalInput").ap()
    wv_d = nc.dram_tensor("wv", (128, 4, G_DIM), bf16, kind="ExternalInput").ap()
    wo_d = nc.dram_tensor("wo", (128, 2, HIDDEN), bf16, kind="ExternalInput").ap()
    qb_d = nc.dram_tensor("qb", (128, 2), f32, kind="ExternalInput").ap()
    kb_d = nc.dram_tensor("kb", (128, 2), f32, kind="ExternalInput").ap()
    mk_d = nc.dram_tensor("mk", (128, NKB), f32, kind="ExternalInput").ap()
    y_d = nc.dram_tensor("y", (N, HIDDEN), f32, kind="ExternalOutput").ap()

    with tile.TileContext(nc) as tc:
        with tc.tile_pool(name="sb", bufs=1) as sb, \
             tc.tile_pool(name="pp", bufs=6) as pp, \
             tc.tile_pool(name="pr", bufs=2) as pr, \
             tc.tile_pool(name="ps", bufs=1, space="PSUM") as ps:
            # ---- persistent SBUF tiles ----
            xt_sb = [sb.tile([128, N], bf16, tag=f"xt{c}", name=f"xt{c}")
                     for c in range(4)]
            wq_sb = sb.tile([128, 4, G_DIM], bf16, tag="wq", name="wq_sb")
            wk_sb = sb.tile([128, 4, G_DIM], bf16, tag="wk", name="wk_sb")
            wv_sb = sb.tile([128, 4, G_DIM], bf16, tag="wv", name="wv_sb")
            wo_sb = sb.tile([128, 2, HIDDEN], bf16, tag="wo", name="wo_sb")
            qb_sb = sb.tile([128, 2], f32, tag="qb", name="qb_sb")
            kb_sb = sb.tile([128, 2], f32, tag="kb", name="kb_sb")
            mk_sb = sb.tile([128, NKB], f32, tag="mk", name="mk_sb")
            qt_sb = [[sb.tile([128, QCH], bf16, tag=f"qt{hp}{h}",
                              name=f"qt{hp}{h}") for h in range(2)]
                     for hp in range(2)]
            kt_sb = [[sb.tile([128, QCH], bf16, tag=f"kt{hp}{h}",
                              name=f"kt{hp}{h}") for h in range(2)]
                     for hp in range(2)]
            # V with a ones column per (key-block, head), split by kb parity
            v_sbs = [sb.tile([128, NKB // 2, G_HEADS, HEAD_DIM + 1], bf16,
                             tag=f"v{par}", name=f"v_sb{par}")
                     for par in range(2)]
            ot_sb = [sb.tile([128, N], bf16, tag=f"ot{hp}", name=f"ot{hp}")
                     for hp in range(2)]

            # ---- input DMAs (need-order: weights for hp0 Q/K first) ----
            warm = sb.tile([1, 4], f32, tag="warm", name="warm")
            nc.vector.memset(warm[:], 0.0)
            nc.scalar.activation(warm[:], warm[:], Exp)  # preload Exp table
            nc.sync.dma_start(wq_sb[:], wq_d[:])
            nc.sync.dma_start(wk_sb[:], wk_d[:])
            for c in range(4):
                nc.sync.dma_start(xt_sb[c][:], xt_d[c * 128:(c + 1) * 128, :])
            nc.gpsimd.dma_start(wv_sb[:], wv_d[:])
            nc.gpsimd.dma_start(qb_sb[:], qb_d[:])
            nc.gpsimd.dma_start(kb_sb[:], kb_d[:])
            nc.gpsimd.dma_start(mk_sb[:], mk_d[:])
            nc.gpsimd.dma_start(wo_sb[:], wo_d[:])
            nc.vector.memset(v_sbs[0][:], 1.0)
            nc.vector.memset(v_sbs[1][:], 1.0)

            # ---- phase A: projections (psum tags shared with attention) ----
            vk = 0   # V key-block emission counter (interleaved with Q/K)

            def emit_v_block():
                nonlocal vk
                if vk >= NKB:
                    return
                k = vk
                psv = ps.tile([128, G_DIM], f32, tag=f"b{k % 2}",
                              name=f"ps_v{k}")
                for c in range(4):
                    nc.tensor.matmul(
                        psv[:],
                        lhsT=xt_sb[c][:, k * 128:(k + 1) * 128],
                        rhs=wv_sb[:, c, :],
                        start=(c == 0), stop=(c == 3))
                nc.vector.tensor_copy(
                    v_sbs[k % 2][:, k // 2, :, 0:HEAD_DIM],
                    psv.rearrange("p (h d) -> p h d", h=G_HEADS))
                vk += 1

            def emit_qk(which, hp, half, tag):
                w_sb, bias_sb, dst = (
                    (wq_sb, qb_sb, qt_sb) if which == "q"
                    else (wk_sb, kb_sb, kt_sb))
                pst = ps.tile([128, QCH], f32, tag=tag,
                              name=f"ps_{which}{hp}{half}")
                for c in range(4):
                    for s in range(2):
                        fr = half * QCH + s * 512
                        nc.tensor.matmul(
                            pst[:, s * 512:(s + 1) * 512],
                            lhsT=w_sb[:, c, hp * 128:(hp + 1) * 128],
                            rhs=xt_sb[c][:, fr:fr + 512],
                            start=(c == 0), stop=(c == 3))
                nc.vector.tensor_scalar_add(
                    dst[hp][half][:], pst[:], bias_sb[:, hp:hp + 1])

            # need-order: first score matmul only waits on (q,hp0,h0)+(k,hp0,h0)
            emit_qk("q", 0, 0, "a0")
            emit_qk("k", 0, 0, "a1")
            emit_v_block(); emit_v_block()
            emit_qk("k", 0, 1, "b0")
            emit_qk("q", 1, 0, "b1")
            emit_qk("k", 1, 0, "b0")
            emit_qk("k", 1, 1, "b1")
            emit_qk("q", 0, 1, "b0")
            emit_qk("q", 1, 1, "b1")
            while vk < NKB:
                emit_v_block()

            # ---- phase B: attention + per-chunk out-projection ----
            for qc in range(2):
                q0 = qc * QCH
                for hp in range(2):
                    b_ps = [ps.tile([HEAD_DIM + 1, QCH], f32, tag=f"b{i}",
                                    name=f"ps_b{qc}{hp}{i}")
                            for i in range(2)]
                    for k in range(NKB):
                        a_ps = [ps.tile([128, QCH], f32, tag=f"a{i}",
                                        name=f"ps_a{qc}{hp}{k}{i}")
                                for i in range(2)]
                        p_t = [pp.tile([128, QCH], bf16, tag=f"p{i}",
                                       name=f"p{qc}{hp}{k}{i}")
                               for i in range(2)]
                        for i in range(2):   # i = head within pair
                            r0 = i * 64
                            for s in range(2):
                                nc.tensor.matmul(
                                    a_ps[i][:, s * 512:(s + 1) * 512],
                                    lhsT=kt_sb[hp][k // 8][r0:r0 + 64,
                                                           (k % 8) * 128:
                                                           (k % 8 + 1) * 128],
                                    rhs=qt_sb[hp][qc][r0:r0 + 64,
                                                      s * 512:(s + 1) * 512],
                                    start=True, stop=True)
                            nc.scalar.activation(
                                p_t[i][:], a_ps[i][:], Exp,
                                bias=mk_sb[:, k:k + 1], scale=SCALE)
                            h = 2 * hp + i
                            for s in range(2):
                                nc.tensor.matmul(
                                    b_ps[i][:, s * 512:(s + 1) * 512],
                                    lhsT=v_sbs[k % 2][:, k // 2, h, :],
                                    rhs=p_t[i][:, s * 512:(s + 1) * 512],
                                    start=(k == 0), stop=(k == NKB - 1))
                    # normalize: reciprocal of denominators (row 64 of b),
                    # partition_broadcast on GPSIMD, multiply into O^T sbuf
                    for i in range(2):
                        r_t = pr.tile([1, QCH], f32, tag=f"r{i}",
                                      name=f"r{qc}{hp}{i}")
                        nc.vector.reciprocal(
                            r_t[:], b_ps[i][HEAD_DIM:HEAD_DIM + 1, :])
                        rb_t = pr.tile([HEAD_DIM, QCH], f32, tag=f"rb{i}",
                                       name=f"rb{qc}{hp}{i}")
                        nc.gpsimd.partition_broadcast(rb_t[:], r_t[:])
                        nc.vector.tensor_mul(
                            ot_sb[hp][i * 64:(i + 1) * 64, q0:q0 + QCH],
                            b_ps[i][0:HEAD_DIM, :], rb_t[:])
                # out-projection for this query chunk (b-slots are free now).
                # Copies alternate DVE/ACT: both are otherwise idle here.
                with tc.tile_pool(name=f"ysb{qc}", bufs=4) as ys:
                    for j in range(NQB // 2):
                        qb = qc * (NQB // 2) + j
                        yp = ps.tile([128, HIDDEN], f32, tag=f"b{j % 2}",
                                     name=f"ps_y{qb}")
                        for hp in range(2):
                            nc.tensor.matmul(
                                yp[:],
                                lhsT=ot_sb[hp][:, qb * 128:(qb + 1) * 128],
                                rhs=wo_sb[:, hp, :],
                                start=(hp == 0), stop=(hp == 1))
                        yt = ys.tile([128, HIDDEN], f32, tag="yt",
                                     name=f"yt{qb}")
                        if j % 2 == 0:
                            nc.vector.tensor_copy(yt[:], yp[:])
                        else:
                            nc.scalar.copy(yt[:], yp[:])
                        nc.sync.dma_start(y_d[qb * 128:(qb + 1) * 128, :],
                                          yt[:])

    nc.compile()
    return nc


def _get_program():
    if "nc" not in _CACHE:
        _CACHE["nc"] = _build_program()
    return _CACHE["nc"]


def _prep_inputs(x, cancer_type, attn_mask, wq, bq, wk, bk, wv, bv, wo, bo,
                 bias_emb, keymod_emb):
    """Host-side shard prep: returns (in_maps list of 8, epilogue (512,))."""
    x = np.asarray(x, dtype=np.float32)
    ct = np.asarray(cancer_type).astype(np.int64)
    mask = np.asarray(attn_mask)
    wq = np.asarray(wq, dtype=np.float32)
    wk = np.asarray(wk, dtype=np.float32)
    wv = np.asarray(wv, dtype=np.float32)
    wo = np.asarray(wo, dtype=np.float32)
    bq = np.asarray(bq, dtype=np.float32)
    bk = np.asarray(bk, dtype=np.float32)
    bv = np.asarray(bv, dtype=np.float32)
    bo = np.asarray(bo, dtype=np.float32)
    keymod = np.asarray(keymod_emb, dtype=np.float32)

    wqt = np.ascontiguousarray(wq.T).astype(BF16)     # (in 512, out 512)
    wkt = np.ascontiguousarray(wk.T).astype(BF16)
    wvt = np.ascontiguousarray(wv.T).astype(BF16)
    wot = np.ascontiguousarray(wo.T).astype(BF16)

    xt_all = [np.ascontiguousarray(x[b].T).astype(BF16) for b in range(B)]
    mka = np.where(mask, np.float32(MASK_NEG), np.float32(0.0)).astype(np.float32)

    in_maps = []
    for core in range(N_CORES):
        b, g = core // 2, core % 2
        gs = slice(g * G_DIM, (g + 1) * G_DIM)
        kbias = np.ascontiguousarray(
            (bk + keymod[ct[b]])[gs].reshape(2, 128).T).astype(np.float32)
        qbias = np.ascontiguousarray(bq[gs].reshape(2, 128).T).astype(np.float32)
        in_maps.append({
            "xt": xt_all[b],
            "wq": np.ascontiguousarray(
                wqt[:, gs].reshape(4, 128, G_DIM).transpose(1, 0, 2)),
            "wk": np.ascontiguousarray(
                wkt[:, gs].reshape(4, 128, G_DIM).transpose(1, 0, 2)),
            "wv": np.ascontiguousarray(
                wvt[:, gs].reshape(4, 128, G_DIM).transpose(1, 0, 2)),
            "wo": np.ascontiguousarray(
                wot[gs, :].reshape(2, 128, HIDDEN).transpose(1, 0, 2)),
            "qb": qbias,
            "kb": kbias,
            "mk": np.ascontiguousarray(mka[b].reshape(NKB, 128).T),
        })
    epilogue = (bv @ wo.T + bo).astype(np.float32)    # (512,)
    return in_maps, epilogue


def kernel(**inputs):
    from concourse import bass_utils

    nc = _get_program()
    in_maps, epilogue = _prep_inputs(**inputs)
    res = bass_utils.run_bass_kernel_spmd(nc, in_maps,
                                          core_ids=list(range(N_CORES)))
    out = np.empty((B, N, HIDDEN), dtype=np.float32)
    for b in range(B):
        out[b] = res.results[2 * b]["y"] + res.results[2 * b + 1]["y"] + epilogue
    return out


# revision 11
# speedup vs baseline: 1.2249x; 1.0059x over previous
"""ContextConditionedAttention Trainium2 kernel.

Full-input contract: kernel(**inputs) takes the unsharded numpy inputs and
returns the full (B, N, HIDDEN) float32 output. Internally the work is
sharded over 8 NeuronCores as (batch b in 0..3) x (head-group g in 0..1),
4 heads per core. Each core computes its head-group's partial out-projection
(2048, 512); the host sums the two head-group partials per batch and adds
the bias epilogue.

Math notes (exact simplifications vs the reference):
  - per-(batch,head) softmax bias bias_emb[ct] is constant along the softmax
    axis -> cancels in softmax -> dropped.
  - keymod_emb[ct] adds to K -> folded into the K projection bias.
  - attn_mask folds into the exp() activation as a per-key additive bias
    (0 or -1e30).
  - V bias + out bias: softmax rows sum to 1 -> P@(V + 1 bv^T) = P@V + 1 bv^T,
    so host epilogue adds (bv @ wo.T + bo).

On-chip layout (per core): everything is computed transposed so no on-chip
transposes are needed:
  Q^T/K^T (d on partitions, tokens free) from lhsT=wq^T chunks, rhs=x^T;
  S^T = K @ Q^T (keys on partitions, row-tiled head pairs); P^T = exp(S^T/8
  + mask) on the scalar engine; O^T accumulated from lhsT=[V | 1] (the ones
  column yields the softmax denominator in row 64); normalization via DVE
  reciprocal + GPSIMD partition_broadcast; out-proj from lhsT=O^T blocks,
  emitted per query-chunk so the store overlaps the next chunk's attention.

PSUM (8 banks total) is a single pool: tags a0/a1 (128,1024 f32, 2 banks
each) shared by Q/K projections, score tiles, and the normalize broadcast;
tags b0/b1 (65,1024 f32, 2 banks each) shared by V projection, O^T
accumulators, and the out-projection tiles.
"""

import numpy as np
import ml_dtypes

B, N, HIDDEN = 4, 2048, 512
N_HEADS, HEAD_DIM = 8, 64
G_HEADS = 4          # heads per core (head-group)
G_DIM = 256          # dims per head-group
N_CORES = 8
NKB = N // 128       # key blocks of 128
NQB = N // 128       # query blocks of 128
QCH = 1024           # query chunk for the attention inner loop
SCALE = 1.0 / float(np.sqrt(HEAD_DIM))
MASK_NEG = -1.0e30

BF16 = ml_dtypes.bfloat16

_CACHE = {}


def _build_program():
    import concourse.bacc as bacc
    import concourse.mybir as mybir
    import concourse.tile as tile
    from concourse import bass_isa

    nc = bacc.Bacc("TRN2", target_bir_lowering=False, debug=False,
                   num_devices=N_CORES)
    f32 = mybir.dt.float32
    bf16 = mybir.dt.bfloat16
    Exp = mybir.ActivationFunctionType.Exp

    # DRAM I/O (per-core shards; same program on all 8 cores)
    xt_d = nc.dram_tensor("xt", (HIDDEN, N), bf16, kind="ExternalInput").ap()
    wq_d = nc.dram_tensor("wq", (128, 4, G_DIM), bf16, kind="ExternalInput").ap()
    wk_d = nc.dram_tensor("wk", (128, 4, G_DIM), bf16, kind="ExternalInput").ap()
    wv_d = nc.dram_tensor("wv", (128, 4, G_DIM), bf16, kind="ExternalInput").ap()
    wo_d = nc.dram_tensor("wo", (128, 2, HIDDEN), bf16, kind="ExternalInput").ap()
    qb_d = nc.dram_tensor("qb", (128, 2), f32, kind="ExternalInput").ap()
    kb_d = nc.dram_tensor("kb", (128, 2), f32, kind="ExternalInput").ap()
    mk_d = nc.dram_tensor("mk", (128, NKB), f32, kind="ExternalInput").ap()
    y_d = nc.dram_tensor("y", (N, HIDDEN), f32, kind="ExternalOutput").ap()

    with tile.TileContext(nc) as tc:
        with tc.tile_pool(name="sb", bufs=1) as sb, \
             tc.tile_pool(name="pp", bufs=8) as pp, \
             tc.tile_pool(name="pr", bufs=2) as pr, \
             tc.tile_pool(name="ps", bufs=1, space="PSUM") as ps:
            # ---- persistent SBUF tiles ----
            xt_sb = [sb.tile([128, N], bf16, tag=f"xt{c}", name=f"xt{c}")
                     for c in range(4)]
            wq_sb = sb.tile([128, 4, G_DIM], bf16, tag="wq", name="wq_sb")
            wk_sb = sb.tile([128, 4, G_DIM], bf16, tag="wk", name="wk_sb")
            wv_sb = sb.tile([128, 4, G_DIM], bf16, tag="wv", name="wv_sb")
            wo_sb = sb.tile([128, 2, HIDDEN], bf16, tag="wo", name="wo_sb")
            qb_sb = sb.tile([128, 2], f32, tag="qb", name="qb_sb")
            kb_sb = sb.tile([128, 2], f32, tag="kb", name="kb_sb")
            mk_sb = sb.tile([128, NKB], f32, tag="mk", name="mk_sb")
            qt_sb = [[sb.tile([128, QCH], bf16, tag=f"qt{hp}{h}",
                              name=f"qt{hp}{h}") for h in range(2)]
                     for hp in range(2)]
            kt_sb = [[sb.tile([128, QCH], bf16, tag=f"kt{hp}{h}",
                              name=f"kt{hp}{h}") for h in range(2)]
                     for hp in range(2)]
            # V with a ones column per (key-block, head), split by kb parity
            v_sbs = [sb.tile([128, NKB // 2, G_HEADS, HEAD_DIM + 1], bf16,
                             tag=f"v{par}", name=f"v_sb{par}")
                     for par in range(2)]
            ot_sb = [sb.tile([128, N], bf16, tag=f"ot{hp}", name=f"ot{hp}")
                     for hp in range(2)]

            # ---- input DMAs (need-order: weights for hp0 Q/K first) ----
            warm = sb.tile([1, 4], f32, tag="warm", name="warm")
            nc.vector.memset(warm[:], 0.0)
            nc.scalar.activation(warm[:], warm[:], Exp)  # preload Exp table
            nc.sync.dma_start(wq_sb[:], wq_d[:])
            nc.sync.dma_start(wk_sb[:], wk_d[:])
            for c in range(4):
                nc.sync.dma_start(xt_sb[c][:], xt_d[c * 128:(c + 1) * 128, :])
            nc.gpsimd.dma_start(wv_sb[:], wv_d[:])
            nc.gpsimd.dma_start(qb_sb[:], qb_d[:])
            nc.gpsimd.dma_start(kb_sb[:], kb_d[:])
            nc.gpsimd.dma_start(mk_sb[:], mk_d[:])
            nc.gpsimd.dma_start(wo_sb[:], wo_d[:])
            nc.vector.memset(v_sbs[0][:], 1.0)
            nc.vector.memset(v_sbs[1][:], 1.0)

            # ---- phase A: projections (psum tags shared with attention) ----
            vk = 0   # V key-block emission counter (interleaved with Q/K)

            def emit_v_block():
                nonlocal vk
                if vk >= NKB:
                    return
                k = vk
                psv = ps.tile([128, G_DIM], f32, tag=f"b{k % 2}",
                              name=f"ps_v{k}")
                for c in range(4):
                    nc.tensor.matmul(
                        psv[:],
                        lhsT=xt_sb[c][:, k * 128:(k + 1) * 128],
                        rhs=wv_sb[:, c, :],
                        start=(c == 0), stop=(c == 3))
                nc.vector.tensor_copy(
                    v_sbs[k % 2][:, k // 2, :, 0:HEAD_DIM],
                    psv.rearrange("p (h d) -> p h d", h=G_HEADS))
                vk += 1

            def emit_qk(which, hp, half, tag):
                w_sb, bias_sb, dst = (
                    (wq_sb, qb_sb, qt_sb) if which == "q"
                    else (wk_sb, kb_sb, kt_sb))
                pst = ps.tile([128, QCH], f32, tag=tag,
                              name=f"ps_{which}{hp}{half}")
                for c in range(4):
                    for s in range(2):
                        fr = half * QCH + s * 512
                        nc.tensor.matmul(
                            pst[:, s * 512:(s + 1) * 512],
                            lhsT=w_sb[:, c, hp * 128:(hp + 1) * 128],
                            rhs=xt_sb[c][:, fr:fr + 512],
                            start=(c == 0), stop=(c == 3))
                nc.vector.tensor_scalar_add(
                    dst[hp][half][:], pst[:], bias_sb[:, hp:hp + 1])

            # need-order: first score matmul only waits on (q,hp0,h0)+(k,hp0,h0)
            emit_qk("q", 0, 0, "a0")
            emit_qk("k", 0, 0, "a1")
            emit_v_block(); emit_v_block()
            emit_qk("k", 0, 1, "b0")
            emit_qk("q", 1, 0, "b1")
            emit_qk("k", 1, 0, "b0")
            emit_qk("k", 1, 1, "b1")
            emit_qk("q", 0, 1, "b0")
            emit_qk("q", 1, 1, "b1")
            while vk < NKB:
                emit_v_block()

            # ---- phase B: attention + per-chunk out-projection ----
            for qc in range(2):
                q0 = qc * QCH
                for hp in range(2):
                    b_ps = [ps.tile([HEAD_DIM + 1, QCH], f32, tag=f"b{i}",
                                    name=f"ps_b{qc}{hp}{i}")
                            for i in range(2)]
                    for k in range(NKB):
                        a_ps = [ps.tile([128, QCH], f32, tag=f"a{i}",
                                        name=f"ps_a{qc}{hp}{k}{i}")
                                for i in range(2)]
                        p_t = [pp.tile([128, QCH], bf16, tag=f"p{i}",
                                       name=f"p{qc}{hp}{k}{i}")
                               for i in range(2)]
                        for i in range(2):   # i = head within pair
                            r0 = i * 64
                            for s in range(2):
                                nc.tensor.matmul(
                                    a_ps[i][:, s * 512:(s + 1) * 512],
                                    lhsT=kt_sb[hp][k // 8][r0:r0 + 64,
                                                           (k % 8) * 128:
                                                           (k % 8 + 1) * 128],
                                    rhs=qt_sb[hp][qc][r0:r0 + 64,
                                                      s * 512:(s + 1) * 512],
                                    start=True, stop=True)
                            nc.scalar.activation(
                                p_t[i][:], a_ps[i][:], Exp,
                                bias=mk_sb[:, k:k + 1], scale=SCALE)
                            h = 2 * hp + i
                            for s in range(2):
                                nc.tensor.matmul(
                                    b_ps[i][:, s * 512:(s + 1) * 512],
                                    lhsT=v_sbs[k % 2][:, k // 2, h, :],
                                    rhs=p_t[i][:, s * 512:(s + 1) * 512],
                                    start=(k == 0), stop=(k == NKB - 1))
                    # normalize: reciprocal of denominators (row 64 of b),
                    # partition_broadcast on GPSIMD, multiply into O^T sbuf
                    for i in range(2):
                        r_t = pr.tile([1, QCH], f32, tag=f"r{i}",
                                      name=f"r{qc}{hp}{i}")
                        nc.vector.reciprocal(
                            r_t[:], b_ps[i][HEAD_DIM:HEAD_DIM + 1, :])
                        rb_t = pr.tile([HEAD_DIM, QCH], f32, tag=f"rb{i}",
                                       name=f"rb{qc}{hp}{i}")
                        nc.gpsimd.partition_broadcast(rb_t[:], r_t[:])
                        nc.vector.tensor_mul(
                            ot_sb[hp][i * 64:(i + 1) * 64, q0:q0 + QCH],
                            b_ps[i][0:HEAD_DIM, :], rb_t[:])
                # out-projection for this query chunk (b-slots are free now).
                # Copies alternate DVE/ACT: both are otherwise idle here.
                with tc.tile_pool(name=f"ysb{qc}", bufs=8) as ys:
                    for j in range(NQB // 2):
                        qb = qc * (NQB // 2) + j
                        yp = ps.tile([128, HIDDEN], f32, tag=f"b{j % 2}",
                                     name=f"ps_y{qb}")
                        for hp in range(2):
                            nc.tensor.matmul(
                                yp[:],
                                lhsT=ot_sb[hp][:, qb * 128:(qb + 1) * 128],
                                rhs=wo_sb[:, hp, :],
                                start=(hp == 0), stop=(hp == 1))
                        yt = ys.tile([128, HIDDEN], f32, tag="yt",
                                     name=f"yt{qb}")
                        if j % 2 == 0:
                            nc.vector.tensor_copy(yt[:], yp[:])
                        else:
                            nc.scalar.copy(yt[:], yp[:])
                        nc.sync.dma_start(y_d[qb * 128:(qb + 1) * 128, :],
                                          yt[:])

    nc.compile()
    return nc


def _get_program():
    if "nc" not in _CACHE:
        _CACHE["nc"] = _build_program()
    return _CACHE["nc"]


def _prep_inputs(x, cancer_type, attn_mask, wq, bq, wk, bk, wv, bv, wo, bo,
                 bias_emb, keymod_emb):
    """Host-side shard prep: returns (in_maps list of 8, epilogue (512,))."""
    x = np.asarray(x, dtype=np.float32)
    ct = np.asarray(cancer_type).astype(np.int64)
    mask = np.asarray(attn_mask)
    wq = np.asarray(wq, dtype=np.float32)
    wk = np.asarray(wk, dtype=np.float32)
    wv = np.asarray(wv, dtype=np.float32)
    wo = np.asarray(wo, dtype=np.float32)
    bq = np.asarray(bq, dtype=np.float32)
    bk = np.asarray(bk, dtype=np.float32)
    bv = np.asarray(bv, dtype=np.float32)
    bo = np.asarray(bo, dtype=np.float32)
    keymod = np.asarray(keymod_emb, dtype=np.float32)

    wqt = np.ascontiguousarray(wq.T).astype(BF16)     # (in 512, out 512)
    wkt = np.ascontiguousarray(wk.T).astype(BF16)
    wvt = np.ascontiguousarray(wv.T).astype(BF16)
    wot = np.ascontiguousarray(wo.T).astype(BF16)

    xt_all = [np.ascontiguousarray(x[b].T).astype(BF16) for b in range(B)]
    mka = np.where(mask, np.float32(MASK_NEG), np.float32(0.0)).astype(np.float32)

    in_maps = []
    for core in range(N_CORES):
        b, g = core // 2, core % 2
        gs = slice(g * G_DIM, (g + 1) * G_DIM)
        kbias = np.ascontiguousarray(
            (bk + keymod[ct[b]])[gs].reshape(2, 128).T).astype(np.float32)
        qbias = np.ascontiguousarray(bq[gs].reshape(2, 128).T).astype(np.float32)
        in_maps.append({
            "xt": xt_all[b],
            "wq": np.ascontiguousarray(
                wqt[:, gs].reshape(4, 128, G_DIM).transpose(1, 0, 2)),
            "wk": np.ascontiguousarray(
                wkt[:, gs].reshape(4, 128, G_DIM).transpose(1, 0, 2)),
            "wv": np.ascontiguousarray(
                wvt[:, gs].reshape(4, 128, G_DIM).transpose(1, 0, 2)),
            "wo": np.ascontiguousarray(
                wot[gs, :].reshape(2, 128, HIDDEN).transpose(1, 0, 2)),
            "qb": qbias,
            "kb": kbias,
            "mk": np.ascontiguousarray(mka[b].reshape(NKB, 128).T),
        })
    epilogue = (bv @ wo.T + bo).astype(np.float32)    # (512,)
    return in_maps, epilogue


def kernel(**inputs):
    from concourse import bass_utils

    nc = _get_program()
    in_maps, epilogue = _prep_inputs(**inputs)
    res = bass_utils.run_bass_kernel_spmd(nc, in_maps,
                                          core_ids=list(range(N_CORES)))
    out = np.empty((B, N, HIDDEN), dtype=np.float32)
    for b in range(B):
        out[b] = res.results[2 * b]["y"] + res.results[2 * b + 1]["y"] + epilogue
    return out
